# revision 1
# baseline (speedup 1.0000x reference)
"""Trainium2 Bass kernel for nn_CNN_RNN_88347477278730.

Pipeline (data-parallel over batch, 8 rows per core on 8 cores):
  kernel1 (device): input projection emb @ Wih_c.T (+biases) hoisted, then the
      512-step select-policy GRUCell recurrence; per step the Gumbel-perturbed
      logit-diff decision k_t = (h . wdiff > -cdiff_t) is emitted.
  host: compaction (gather kept tokens to the front), new_lens, Ldyn, masks.
  kernel2 (device): proj of compacted embeddings, 2-layer GRU recurrences,
      valid-masking, Kim-CNN convs as shifted matmuls, masked max-pool, final
      linear.

All matmul layouts are "weights stationary": lhsT = weight tiles
[K=128, M=128], moving operand = transposed activations [K, batch], so
gate tensors land partition-major ([128 gate dims, small free]) where the
elementwise engines are fast.
"""

import os
import subprocess
import sys
import tempfile

import numpy as np

# ---------------------------------------------------------------- constants
B, T, E, H, NF = 64, 512, 768, 256, 100
NCORES = 8
BPC = B // NCORES  # batch rows per core
KE = E // 128      # 6 K-tiles over the embedding dim
KH = H // 128      # 2 K-tiles over the hidden dim
GC = (3 * H) // 128  # 6 gate chunks (r: 0-1, z: 2-3, n: 4-5)
NEG = -1.0e30

_F32 = None  # set lazily to mybir.dt.float32


# ------------------------------------------------------------- tile patch
def _apply_tile_patch():
    """This walrus build rejects >2 sem waits on one SP control instruction;
    split the TileContext tail drain into several drains of <=2 waits."""
    import concourse.tile as tile
    from concourse.vector_clock import ScopedClock, VectorClock

    if getattr(tile.TileContext, "_drain_split_patched", False):
        return

    def _patched(self, tick_clock, wait_clock):
        gc = tick_clock.global_clock
        n = len(gc)
        for start in range(0, n, 1):
            vec = [0] * n
            any_set = False
            for p in range(start, min(start + 1, n)):
                vec[p] = gc[p]
                any_set = any_set or vec[p] > 0
            if not any_set:
                continue
            d = self.nc.sync.drain()
            wait_clock.add_sem_waits(d.ins, ScopedClock({None: VectorClock(vec)}))
        self.nc.all_engine_barrier()
        assert self.sems is not None
        popped = self.nc._tile_sem_poison_stack.pop()
        assert popped is self._sem_poison
        self.nc.clear_and_free_semaphores(list(self.sems.allocated().values()))
        self.nc.all_engine_barrier()

    tile.TileContext._drain_and_barrier = _patched
    tile.TileContext._drain_split_patched = True


# ------------------------------------------------------------- gumbel (CPU)
def _gumbel_cpu():
    """jax.random.gumbel(key(42), (T-1, B, 2), f32) — computed in a CPU-jax
    subprocess so the accelerator backend is never involved (it must be
    bit-identical to the reference's CPU computation)."""
    path = os.path.join(tempfile.mkdtemp(), "gumbel.npy")
    code = (
        "import numpy as np, jax, jax.numpy as jnp\n"
        f"g = jax.random.gumbel(jax.random.key(42), ({T - 1}, {B}, 2), jnp.float32)\n"
        f"np.save({path!r}, np.asarray(g))\n"
    )
    env = dict(os.environ)
    env["TRN_TERMINAL_POOL_IPS"] = ""
    env["JAX_PLATFORMS"] = "cpu"
    extra = [p for p in sys.path if p and os.path.isdir(p)]
    env["PYTHONPATH"] = os.pathsep.join(extra)
    subprocess.run([sys.executable, "-c", code], env=env, check=True, capture_output=True)
    return np.load(path)


# ------------------------------------------------------------- host packing
def _pack_T(a2d):
    """[rows(=128*k), cols] -> [128, k, cols] weight-tile layout."""
    rows, cols = a2d.shape
    k = rows // 128
    return np.ascontiguousarray(a2d.reshape(k, 128, cols).transpose(1, 0, 2)).astype(np.float32)


def _pack_bias(b1d):
    """[128*k] -> [128, k]"""
    k = b1d.shape[0] // 128
    return np.ascontiguousarray(b1d.reshape(k, 128).T).astype(np.float32)


def _pack_embT(emb_rows, t_len=T):
    """[bpc, T, E] -> [KE, 128, bpc*T] (e-major tiles, free dims (b, t))."""
    bpc = emb_rows.shape[0]
    x = emb_rows.transpose(2, 0, 1).reshape(KE, 128, bpc * t_len)
    return np.ascontiguousarray(x).astype(np.float32)


def _pack_gru_weights(Wih, Whh, bih, bhh, extra_col=None):
    """Returns (wihT, whhT, bias_proj, bhhn_rep) packings.

    bias_proj folds bih+bhh for the r,z chunks (added once at projection
    time); n chunks get bih only, with bhh_n applied per-step (it must be
    added to h@Whh_n *before* the r* multiply).
    """
    wihT = _pack_T(np.ascontiguousarray(Wih.T))  # [128, KE or KH, 3H]
    wp = np.ascontiguousarray(Whh.T)  # [H, 3H]
    if extra_col is not None:
        wp = np.concatenate([wp, extra_col[:, None]], axis=1)  # [H, 3H+1]
    whhT = _pack_T(wp)  # [128, KH, 3H(+1)]
    bias = np.empty(3 * H, np.float32)
    bias[: 2 * H] = bih[: 2 * H] + bhh[: 2 * H]
    bias[2 * H :] = bih[2 * H :]
    bias_proj = _pack_bias(bias)  # [128, GC]
    bhhn = _pack_bias(bhh[2 * H :])  # [128, KH]
    bhhn_rep = np.ascontiguousarray(
        np.broadcast_to(bhhn[:, :, None], (128, KH, BPC))
    ).astype(np.float32)
    return wihT, whhT, bias_proj, bhhn_rep


# ------------------------------------------------------------- bass builders
def _mk_nc():
    import concourse.bass as bass

    return bass.Bass("TRN2", target_bir_lowering=False, debug=False, num_devices=1)


def _split_excess_waits(nc, max_waits=1):
    """This walrus build can only encode ~2 sem waits per instruction
    (setupSyncWait 'Too many sync wait commands'). Hoist excess waits onto
    same-engine NoOps inserted just before the over-subscribed instruction;
    engine queues execute in order, so the wait semantics are identical."""
    from concourse import mybir

    nid = [0]
    for f in nc.m.functions:
        for bb in f.blocks:
            out = []
            changed = False
            for inst in bb.instructions:
                si = inst.sync_info
                lim = max_waits
                if si is not None and si.on_wait and len(si.on_wait) > lim:
                    waits = list(si.on_wait)
                    extra, keep = waits[:-lim], waits[-lim:]
                    for j in range(0, len(extra), max_waits):
                        nop = mybir.InstNoOp(
                            name=f"I-waitnop-{nid[0]}", ins=[], outs=[])
                        nid[0] += 1
                        nop.engine = inst.engine
                        nop.sync_info = mybir.SyncInfo(
                            on_wait=extra[j : j + max_waits], on_update=[])
                        nc.register_instruction(nop, overwrite=True)
                        out.append(nop)
                    inst.sync_info = mybir.SyncInfo(
                        on_wait=keep, on_update=list(si.on_update or []))
                    changed = True
                out.append(inst)
            if changed:
                bb.instructions = out
    return nc


def build_kernel1(t_len=T):
    """Select-policy kernel: fp32r input projection, fp16 gh matmuls with an
    fp32 elementwise chain (dual h write), two 4-row batch streams
    interleaved so each stream's serial chain hides behind the other's PE
    work, and decisions batched into one matmul + is_gt pass at the end."""
    import concourse.tile as tile
    from concourse import mybir

    _apply_tile_patch()
    nc = _mk_nc()
    f32 = mybir.dt.float32
    f32r = mybir.dt.float32r
    f16 = mybir.dt.float16
    act = mybir.ActivationFunctionType
    alu = mybir.AluOpType
    SW = BPC // 2  # stream width (rows per stream)

    embT_d = nc.dram_tensor("embT", [KE, 128, BPC * t_len], f32r, kind="ExternalInput").ap()
    wihcT_d = nc.dram_tensor("wihcT", [128, KE, 3 * H], f32r, kind="ExternalInput").ap()
    biasc_d = nc.dram_tensor("biasc", [128, GC], f32, kind="ExternalInput").ap()
    whh16_d = nc.dram_tensor("whh16", [128, KH, 3 * H], f16, kind="ExternalInput").ap()
    bhhnc_d = nc.dram_tensor("bhhnc", [128, KH, BPC], f32, kind="ExternalInput").ap()
    wdiffT_d = nc.dram_tensor("wdiffT", [128, KH, 1], f32, kind="ExternalInput").ap()
    ncdiff_d = nc.dram_tensor("ncdiff", [1, BPC * t_len], f32, kind="ExternalInput").ap()
    ks_d = nc.dram_tensor("ks", [1, BPC * t_len], f32, kind="ExternalOutput").ap()

    with tile.TileContext(nc) as tc:
        from contextlib import ExitStack

        with ExitStack() as ctx:
            wpool = ctx.enter_context(tc.tile_pool(name="weights", bufs=1))
            gipool = ctx.enter_context(tc.tile_pool(name="gi", bufs=1))
            hpool = ctx.enter_context(tc.tile_pool(name="hist", bufs=1))
            dma_pool = ctx.enter_context(tc.tile_pool(name="dma", bufs=2))
            big_ps = ctx.enter_context(tc.tile_pool(name="bigps", bufs=2, space="PSUM"))
            rzA_ps = ctx.enter_context(tc.tile_pool(name="rzAps", bufs=1, space="PSUM"))
            nA_ps = ctx.enter_context(tc.tile_pool(name="nAps", bufs=1, space="PSUM"))
            rzB_ps = ctx.enter_context(tc.tile_pool(name="rzBps", bufs=1, space="PSUM"))
            nB_ps = ctx.enter_context(tc.tile_pool(name="nBps", bufs=1, space="PSUM"))
            sb_pool = ctx.enter_context(tc.tile_pool(name="gates", bufs=2))
            h16_pool = ctx.enter_context(tc.tile_pool(name="h16", bufs=2))
            misc = ctx.enter_context(tc.tile_pool(name="misc", bufs=1))

            def _load(pool, dram, shape, tag, dt=f32):
                t_ = pool.tile(shape, dt, tag=tag)
                nc.sync.dma_start(t_[:], dram[:])
                return t_

            wihcT = _load(wpool, wihcT_d, [128, KE, 3 * H], "wihcT", f32r)
            biasc = _load(wpool, biasc_d, [128, GC], "biasc")
            whh16 = _load(wpool, whh16_d, [128, KH, 3 * H], "whh16", f16)
            bhhnc_rep = _load(wpool, bhhnc_d, [128, KH, BPC], "bhhnc")
            wdiffT = _load(misc, wdiffT_d, [128, KH, 1], "wdiffT")
            ncdiff = _load(misc, ncdiff_d, [1, BPC * t_len], "ncdiff")

            # ---- projection: gi = emb @ Wih_c.T + bias (fp32r full-rate) ----
            gi = gipool.tile([128, t_len, GC, BPC], f32, tag="gi")
            for b in range(BPC):
                src_sb = dma_pool.tile([128, KE, t_len], f32r, tag="projsrc")
                for k in range(KE):
                    nc.sync.dma_start(
                        src_sb[:, k, :], embT_d[k, :, b * t_len : (b + 1) * t_len]
                    )
                for c in range(GC):
                    ps = big_ps.tile([128, t_len], f32, tag="projps")
                    for k in range(KE):
                        nc.tensor.matmul(
                            ps[:],
                            wihcT[:, k, c * 128 : (c + 1) * 128],
                            src_sb[:, k, :],
                            start=(k == 0),
                            stop=(k == KE - 1),
                        )
                    nc.scalar.activation(
                        gi[:, :, c, b], ps[:], act.Identity, bias=biasc[:, c : c + 1]
                    )

            hist = hpool.tile([128, KH, BPC, t_len], f32, tag="hist")
            rzA = rzA_ps.tile([128, 4, SW], f32, tag="rzA")
            nA = nA_ps.tile([128, KH, SW], f32, tag="nA")
            rzB = rzB_ps.tile([128, 4, SW], f32, tag="rzB")
            nB = nB_ps.tile([128, KH, SW], f32, tag="nB")
            ps_map = {"a": (rzA, nA), "b": (rzB, nB)}
            h16_prev = {"a": None, "b": None}

            def emit_step(t, b0, sfx):
                rz_ps, n_ps = ps_map[sfx]
                bs = slice(b0, b0 + SW)
                h16p = h16_prev[sfx]
                rz = sb_pool.tile([128, 4, SW], f32, tag="rz" + sfx)
                tmp = sb_pool.tile([128, KH, SW], f32, tag="tmp" + sfx)
                if h16p is not None:
                    for c in (0, 1, 4, 5, 2, 3):
                        dst = rz_ps[:, c, :] if c < 4 else n_ps[:, c - 4, :]
                        for k in range(KH):
                            nc.tensor.matmul(
                                dst,
                                whh16[:, k, c * 128 : (c + 1) * 128],
                                h16p[:, k, :],
                                start=(k == 0),
                                stop=(k == KH - 1),
                            )
                    nc.vector.tensor_tensor(rz[:], rz_ps[:], gi[:, t, 0:4, bs], alu.add)
                    nc.scalar.activation(rz[:], rz[:], act.Sigmoid)
                    nc.vector.tensor_tensor(tmp[:], n_ps[:], bhhnc_rep[:, :, bs], alu.add)
                else:
                    nc.scalar.activation(rz[:], gi[:, t, 0:4, bs], act.Sigmoid)
                    nc.vector.tensor_copy(tmp[:], bhhnc_rep[:, :, bs])
                nc.vector.tensor_tensor(tmp[:], tmp[:], rz[:, 0:KH, :], alu.mult)
                nc.gpsimd.tensor_tensor(tmp[:], tmp[:], gi[:, t, 4:GC, bs], alu.add)
                nn_ = sb_pool.tile([128, KH, SW], f32, tag="nn" + sfx)
                nc.scalar.activation(nn_[:], tmp[:], act.Tanh)
                d = sb_pool.tile([128, KH, SW], f32, tag="dd" + sfx)
                if t > 0:
                    nc.gpsimd.tensor_tensor(d[:], hist[:, :, bs, t - 1], nn_[:], alu.subtract)
                else:
                    nc.gpsimd.tensor_scalar(d[:], nn_[:], -1.0, None, alu.mult)
                nc.vector.tensor_tensor(d[:], rz[:, 2:4, :], d[:], alu.mult)
                nc.vector.tensor_tensor(hist[:, :, bs, t], nn_[:], d[:], alu.add)
                h16 = h16_pool.tile([128, KH, SW], f16, tag="h16" + sfx)
                nc.gpsimd.tensor_copy(h16[:], hist[:, :, bs, t])
                h16_prev[sfx] = h16

            for t in range(t_len):
                emit_step(t, 0, "a")
                emit_step(t, SW, "b")

            # ---- batched decisions: ks[b,t] = (h_t . wdiff > ncdiff) ----
            ks_sb = misc.tile([1, BPC * t_len], f32, tag="kssb")
            for b in range(BPC):
                dps = big_ps.tile([1, t_len], f32, tag="projps")
                for k in range(KH):
                    nc.tensor.matmul(
                        dps[:],
                        wdiffT[:, k, :],
                        hist[:, k, b, :],
                        start=(k == 0),
                        stop=(k == KH - 1),
                    )
                nc.vector.tensor_tensor(
                    ks_sb[0:1, b * t_len : (b + 1) * t_len], dps[:],
                    ncdiff[0:1, b * t_len : (b + 1) * t_len], alu.is_gt,
                )
            nc.sync.dma_start(ks_d[:], ks_sb[:])

    return _split_excess_waits(nc)


def build_kernel2(t_len=T, debug=False):
    """GRU0/GRU1 + convs + pooling + final linear. Returns nc.

    bf16 matmul operands throughout (2e-2 output tolerance); layer0 and
    layer1 recurrences software-pipelined with a D-step lag so each
    layer's serial gate chain hides behind the other layer's PE work;
    gi1 computed per-chunk from the o1 history; gi_rz and bhh_n are
    preloaded into PSUM on the (otherwise idle) Pool engine so the gate
    matmuls accumulate straight onto them."""
    import concourse.tile as tile
    from concourse import mybir

    _apply_tile_patch()
    nc = _mk_nc()
    f32 = mybir.dt.float32
    bf16 = mybir.dt.float16
    act = mybir.ActivationFunctionType
    alu = mybir.AluOpType
    D = 64
    NCH = t_len // D

    nembT_d = nc.dram_tensor("nembT", [KE, 128, BPC * t_len], bf16, kind="ExternalInput").ap()
    wih0T_d = nc.dram_tensor("wih0T", [128, KE, 3 * H], bf16, kind="ExternalInput").ap()
    whh0T_d = nc.dram_tensor("whh0T", [128, KH, 3 * H], bf16, kind="ExternalInput").ap()
    bias0_d = nc.dram_tensor("bias0", [128, GC], f32, kind="ExternalInput").ap()
    bhhn0_d = nc.dram_tensor("bhhn0", [128, KH, BPC], f32, kind="ExternalInput").ap()
    wih1T_d = nc.dram_tensor("wih1T", [128, KH, 3 * H], bf16, kind="ExternalInput").ap()
    whh1T_d = nc.dram_tensor("whh1T", [128, KH, 3 * H], bf16, kind="ExternalInput").ap()
    bias1_d = nc.dram_tensor("bias1", [128, GC], f32, kind="ExternalInput").ap()
    bhhn1_d = nc.dram_tensor("bhhn1", [128, KH, BPC], f32, kind="ExternalInput").ap()
    vt_d = nc.dram_tensor("vt", [1, BPC * t_len], bf16, kind="ExternalInput").ap()
    cw_d = nc.dram_tensor("cw", [128, 12, KH, NF], bf16, kind="ExternalInput").ap()
    cb_d = nc.dram_tensor("cb", [NF, 3], f32, kind="ExternalInput").ap()
    tmask_d = nc.dram_tensor("tmask", [NF, 3, t_len], f32, kind="ExternalInput").ap()
    woutT_d = nc.dram_tensor("woutT", [NF, 3], f32, kind="ExternalInput").ap()
    bout_d = nc.dram_tensor("bout", [1, 1], f32, kind="ExternalInput").ap()
    out_d = nc.dram_tensor("out", [1, BPC], f32, kind="ExternalOutput").ap()
    if debug:
        gi0dbg_d = nc.dram_tensor("gi0dbg", [128, t_len, GC, BPC], bf16, kind="ExternalOutput").ap()
        gi1dbg_d = nc.dram_tensor("gi1dbg", [128, t_len, GC, BPC], bf16, kind="ExternalOutput").ap()
        o1dbg_d = nc.dram_tensor("o1dbg", [128, KH, BPC, t_len], bf16, kind="ExternalOutput").ap()
        o2dbg_d = nc.dram_tensor("o2dbg", [128, KH, BPC, t_len], bf16, kind="ExternalOutput").ap()
        pooldbg_d = nc.dram_tensor("pooldbg", [NF, 3, BPC], f32, kind="ExternalOutput").ap()

    FS = (3, 4, 5)

    with tile.TileContext(nc) as tc:
        from contextlib import ExitStack

        with ExitStack() as ctx:
            wpool = ctx.enter_context(tc.tile_pool(name="weights", bufs=1))
            gipool = ctx.enter_context(tc.tile_pool(name="gi", bufs=1))
            opool = ctx.enter_context(tc.tile_pool(name="obuf", bufs=1))
            dma_pool = ctx.enter_context(tc.tile_pool(name="dma", bufs=2))
            big_ps = ctx.enter_context(tc.tile_pool(name="bigps", bufs=2, space="PSUM"))
            rz0_ps = ctx.enter_context(tc.tile_pool(name="rz0ps", bufs=1, space="PSUM"))
            n0_ps = ctx.enter_context(tc.tile_pool(name="n0ps", bufs=1, space="PSUM"))
            rz1_ps = ctx.enter_context(tc.tile_pool(name="rz1ps", bufs=1, space="PSUM"))
            n1_ps = ctx.enter_context(tc.tile_pool(name="n1ps", bufs=1, space="PSUM"))
            sb_pool = ctx.enter_context(tc.tile_pool(name="gates", bufs=2))
            misc = ctx.enter_context(tc.tile_pool(name="misc", bufs=1))

            def _load(pool, dram, shape, tag, dt=f32):
                t_ = pool.tile(shape, dt, tag=tag)
                nc.sync.dma_start(t_[:], dram[:])
                return t_

            wih0T = _load(wpool, wih0T_d, [128, KE, 3 * H], "wih0", bf16)
            whh0T = _load(wpool, whh0T_d, [128, KH, 3 * H], "whh0", bf16)
            bias0 = _load(wpool, bias0_d, [128, GC], "bias0")
            bhhn0 = _load(wpool, bhhn0_d, [128, KH, BPC], "bhhn0")
            wih1T = _load(wpool, wih1T_d, [128, KH, 3 * H], "wih1", bf16)
            whh1T = _load(wpool, whh1T_d, [128, KH, 3 * H], "whh1", bf16)
            bias1 = _load(wpool, bias1_d, [128, GC], "bias1")
            bhhn1 = _load(wpool, bhhn1_d, [128, KH, BPC], "bhhn1")
            cw = _load(wpool, cw_d, [128, 12, KH, NF], "cw", bf16)
            cb = _load(misc, cb_d, [NF, 3], "cb")
            tmask = _load(misc, tmask_d, [NF, 3, t_len], "tmask")
            woutT = _load(misc, woutT_d, [NF, 3], "woutT")
            bout = _load(misc, bout_d, [1, 1], "bout")
            vt = _load(misc, vt_d, [1, BPC * t_len], "vt", bf16)

            # ---- proj0: gi0 = (nemb @ Wih0.T) + bias0, bf16 ----
            gi0 = gipool.tile([128, t_len, GC, BPC], bf16, tag="gi0")
            for b in range(BPC):
                src_sb = dma_pool.tile([128, KE, t_len], bf16, tag="projsrc")
                for k in range(KE):
                    nc.sync.dma_start(
                        src_sb[:, k, :], nembT_d[k, :, b * t_len : (b + 1) * t_len]
                    )
                for c in range(GC):
                    ps = big_ps.tile([128, t_len], f32, tag="projps")
                    for k in range(KE):
                        nc.tensor.matmul(
                            ps[:],
                            wih0T[:, k, c * 128 : (c + 1) * 128],
                            src_sb[:, k, :],
                            start=(k == 0),
                            stop=(k == KE - 1),
                        )
                    nc.scalar.activation(
                        gi0[:, :, c, b], ps[:], act.Identity, bias=bias0[:, c : c + 1]
                    )

            gi1 = gipool.tile([128, t_len, GC, BPC], bf16, tag="gi1")
            o1 = opool.tile([128, KH, BPC, t_len], bf16, tag="o1")
            o2 = opool.tile([128, KH, BPC, t_len], bf16, tag="o2")

            rz0 = rz0_ps.tile([128, 4, BPC], f32, tag="rz0")
            n0 = n0_ps.tile([128, KH, BPC], f32, tag="n0")
            rz1 = rz1_ps.tile([128, 4, BPC], f32, tag="rz1")
            n1 = n1_ps.tile([128, KH, BPC], f32, tag="n1")

            h32_prev = {"0": None, "1": None}
            h32_pool = ctx.enter_context(tc.tile_pool(name="h32", bufs=2))

            def emit_step(t, gi, whh, bhhn, hist, rz_ps, n_ps, sfx):
                """One pipelined GRU step; h_t goes to hist (fp16, feeds the
                matmuls/convs) AND to a fp32 ping-pong tile (feeds the next
                step's elementwise chain, so rounding noise doesn't compound
                through the recurrence)."""
                h16_prev = hist[:, :, :, t - 1] if t > 0 else None
                h32p = h32_prev[sfx]
                rz = sb_pool.tile([128, 4, BPC], f32, tag="rz" + sfx)
                tmp = sb_pool.tile([128, KH, BPC], f32, tag="tmp" + sfx)
                if h16_prev is not None:
                    for c in (0, 1, 4, 5, 2, 3):
                        dst = rz_ps[:, c, :] if c < 4 else n_ps[:, c - 4, :]
                        for k in range(KH):
                            nc.tensor.matmul(
                                dst,
                                whh[:, k, c * 128 : (c + 1) * 128],
                                h16_prev[:, k, :],
                                start=(k == 0),
                                stop=(k == KH - 1),
                            )
                    nc.vector.tensor_tensor(rz[:], rz_ps[:], gi[:, t, 0:4, :], alu.add)
                    nc.scalar.activation(rz[:], rz[:], act.Sigmoid)
                    nc.vector.tensor_tensor(tmp[:], n_ps[:], bhhn[:], alu.add)
                else:
                    nc.scalar.activation(rz[:], gi[:, t, 0:4, :], act.Sigmoid)
                    nc.vector.tensor_copy(tmp[:], bhhn[:])
                nc.vector.tensor_tensor(tmp[:], tmp[:], rz[:, 0:KH, :], alu.mult)
                nc.gpsimd.tensor_tensor(tmp[:], tmp[:], gi[:, t, 4:GC, :], alu.add)
                nn_ = sb_pool.tile([128, KH, BPC], f32, tag="nn" + sfx)
                nc.scalar.activation(nn_[:], tmp[:], act.Tanh)
                d = sb_pool.tile([128, KH, BPC], f32, tag="dd" + sfx)
                if h16_prev is not None:
                    nc.gpsimd.tensor_tensor(d[:], h16_prev, nn_[:], alu.subtract)
                else:
                    nc.gpsimd.tensor_scalar(d[:], nn_[:], -1.0, None, alu.mult)
                nc.vector.tensor_tensor(d[:], rz[:, 2:4, :], d[:], alu.mult)
                nc.vector.tensor_tensor(hist[:, :, :, t], nn_[:], d[:], alu.add)

            # ---- pipelined L0 / proj1-chunks / L1 (lag D steps) ----
            for ci in range(NCH + 1):
                for j in range(D):
                    if ci < NCH:
                        emit_step(ci * D + j, gi0, whh0T, bhhn0, o1, rz0, n0, "0")
                    if ci > 0:
                        emit_step((ci - 1) * D + j, gi1, whh1T, bhhn1, o2, rz1, n1, "1")
                if ci < NCH:
                    t0, t1 = ci * D, (ci + 1) * D
                    for b in range(BPC):
                        for c in range(GC):
                            ps = big_ps.tile([128, D], f32, tag="projps")
                            for k in range(KH):
                                nc.tensor.matmul(
                                    ps[:],
                                    wih1T[:, k, c * 128 : (c + 1) * 128],
                                    o1[:, k, b, t0:t1],
                                    start=(k == 0),
                                    stop=(k == KH - 1),
                                )
                            nc.scalar.activation(
                                gi1[:, t0:t1, c, b], ps[:], act.Identity,
                                bias=bias1[:, c : c + 1],
                            )

            # ---- zero o2 past new_lens: o2 *= vt ----
            # partition-broadcast vt via a K=1 ones-matmul (PE outer product)
            ones_sb = misc.tile([1, 128], bf16, tag="ones")
            nc.vector.memset(ones_sb[:], 1.0)
            for b in range(BPC):
                vtb = big_ps.tile([128, t_len], f32, tag="projps")
                nc.tensor.matmul(
                    vtb[:], ones_sb[:], vt[0:1, b * t_len : (b + 1) * t_len],
                    start=True, stop=True,
                )
                for k in range(KH):
                    nc.vector.tensor_tensor(
                        o2[:, k, b, :], o2[:, k, b, :], vtb[:], alu.mult
                    )

            # ---- convs + relu + tmask + max-pool ----
            pooled = misc.tile([NF, 3, BPC], f32, tag="pooled")
            for b in range(BPC):
                for fi, fs in enumerate(FS):
                    nw = t_len - fs + 1
                    ps = big_ps.tile([NF, t_len], f32, tag="projps")
                    m0 = sum(FS[:fi])  # flat (fs,dt) base index
                    first = True
                    for dt in range(fs):
                        for k in range(KH):
                            nc.tensor.matmul(
                                ps[:, :nw],
                                cw[:, m0 + dt, k, :],
                                o2[:, k, b, dt : dt + nw],
                                start=first,
                                stop=(dt == fs - 1 and k == KH - 1),
                            )
                            first = False
                    crelu = sb_pool.tile([NF, t_len], f32, tag="crelu")
                    nc.scalar.activation(
                        crelu[:, :nw], ps[:, :nw], act.Relu, bias=cb[:, fi : fi + 1]
                    )
                    nc.vector.tensor_tensor(
                        crelu[:, :nw], crelu[:, :nw], tmask[:, fi, :nw], alu.add
                    )
                    nc.vector.tensor_reduce(
                        pooled[:, fi, b : b + 1], crelu[:, :nw], mybir.AxisListType.X, alu.max
                    )

            # ---- final linear ----
            fps = big_ps.tile([1, BPC], f32, tag="projps")
            for fi in range(3):
                nc.tensor.matmul(
                    fps[:],
                    woutT[:, fi : fi + 1],
                    pooled[:, fi, :],
                    start=(fi == 0),
                    stop=(fi == 2),
                )
            out_sb = misc.tile([1, BPC], f32, tag="outsb")
            nc.scalar.activation(out_sb[:], fps[:], act.Identity, bias=bout[0:1, 0:1])
            nc.sync.dma_start(out_d[:], out_sb[:])
            if debug:
                nc.sync.dma_start(gi0dbg_d[:], gi0[:])
                nc.sync.dma_start(gi1dbg_d[:], gi1[:])
                nc.sync.dma_start(o1dbg_d[:], o1[:])
                nc.sync.dma_start(o2dbg_d[:], o2[:])
                nc.sync.dma_start(pooldbg_d[:], pooled[:])

    return _split_excess_waits(nc)


# ------------------------------------------------------------- host orchestration
def _host_pack_k1(inputs, gumbel, t_len=T):
    f16 = _np_f16()
    emb = np.asarray(inputs["embedded"], np.float32)
    mask = np.asarray(inputs["mask"])
    lens = mask.sum(1)
    maxlen = int(lens.max())

    wihcT, whhT, biasc, bhhnc = _pack_gru_weights(
        inputs["Wih_c"], inputs["Whh_c"], inputs["bih_c"], inputs["bhh_c"])
    wdiff = (inputs["Wsel"][1] - inputs["Wsel"][0]).astype(np.float32)
    wdiffT = np.ascontiguousarray(wdiff.reshape(KH, 128).T[:, :, None])
    bdiff = float(inputs["bsel"][1] - inputs["bsel"][0])

    # ncdiff[b, t]: k_t = (h.wdiff > ncdiff); forced off when t >= maxlen-1
    ncdiff = np.full((B, t_len), 1.0e30, np.float32)
    upto = min(maxlen - 1, t_len)
    for t in range(1, upto):
        ncdiff[:, t] = -(bdiff + gumbel[t - 1, :, 1] - gumbel[t - 1, :, 0])

    in_maps = []
    for c in range(NCORES):
        rows = slice(c * BPC, (c + 1) * BPC)
        in_maps.append({
            "embT": _pack_embT(emb[rows, :t_len], t_len),
            "wihcT": wihcT,
            "biasc": biasc,
            "whh16": whhT.astype(f16),
            "bhhnc": bhhnc,
            "wdiffT": wdiffT,
            "ncdiff": np.ascontiguousarray(
                ncdiff[rows].reshape(1, BPC * t_len)),
        })
    return in_maps, lens, maxlen


def _host_compact(inputs, ks_full, lens, maxlen, t_len=T):
    """ks_full: [B, t_len] decision bits (row t=0 ignored; selected[:,0]=1)."""
    emb = np.asarray(inputs["embedded"], np.float32)
    selected = np.zeros((B, t_len), np.int64)
    selected[:, 0] = 1
    selected[:, 1:] = ks_full[:, 1:]
    pos = np.arange(t_len)
    sel_valid = np.where(pos[None, :] < (lens - 1)[:, None], selected, 0)
    new_mask = np.where(pos[None, :] == (lens - 1)[:, None], 1, sel_valid)
    new_lens = new_mask.sum(1)
    Ldyn = max(int(new_lens.max()), 7)

    new_emb = np.zeros((B, t_len, E), np.float32)
    for b in range(B):
        idx = np.nonzero(new_mask[b])[0]
        new_emb[b, : len(idx)] = emb[b, idx]
    return new_emb, new_lens, Ldyn


def _np_f16():
    from concourse import mybir

    return mybir.dt.np(mybir.dt.float16)


def _host_pack_k2(inputs, new_emb, new_lens, Ldyn, t_len=T):
    bf16 = _np_f16()
    wih0T, whh0T, bias0, bhhn0 = _pack_gru_weights(
        inputs["Wih0"], inputs["Whh0"], inputs["bih0"], inputs["bhh0"])
    wih1T, whh1T, bias1, bhhn1 = _pack_gru_weights(
        inputs["Wih1"], inputs["Whh1"], inputs["bih1"], inputs["bhh1"])


    FS = (3, 4, 5)
    cw = np.zeros((128, 12, KH, NF), np.float32)
    cb = np.zeros((NF, 3), np.float32)
    m = 0
    for fi, fs in enumerate(FS):
        w = np.asarray(inputs[f"conv_w{fs}"], np.float32)  # [NF,1,fs,H]
        cb[:, fi] = np.asarray(inputs[f"conv_b{fs}"], np.float32)
        for dt in range(fs):
            wt = w[:, 0, dt, :].T  # [H, NF]
            cw[:, m, :, :] = wt.reshape(KH, 128, NF).transpose(1, 0, 2)
            m += 1

    tmask = np.full((NF, 3, t_len), NEG, np.float32)
    for fi, fs in enumerate(FS):
        kf = min(Ldyn - fs + 1, t_len - fs + 1)
        if kf > 0:
            tmask[:, fi, :kf] = 0.0

    woutT = np.ascontiguousarray(
        np.asarray(inputs["Wout"], np.float32)[0].reshape(3, NF).T)
    bout = np.asarray(inputs["bout"], np.float32).reshape(1, 1)

    vt_full = (np.arange(t_len)[None, :] < new_lens[:, None]).astype(np.float32)

    in_maps = []
    for c in range(NCORES):
        rows = slice(c * BPC, (c + 1) * BPC)
        in_maps.append({
            "nembT": _pack_embT(new_emb[rows, :t_len], t_len).astype(bf16),
            "wih0T": wih0T.astype(bf16), "whh0T": whh0T.astype(bf16),
            "bias0": bias0, "bhhn0": bhhn0,
            "wih1T": wih1T.astype(bf16), "whh1T": whh1T.astype(bf16),
            "bias1": bias1, "bhhn1": bhhn1,
            "vt": np.ascontiguousarray(
                vt_full[rows].reshape(1, BPC * t_len)).astype(bf16),
            "cw": cw.astype(bf16), "cb": cb, "tmask": tmask,
            "woutT": woutT, "bout": bout,
        })
    return in_maps


_NC_CACHE = {}


def _get_nc(which, t_len=T):
    key = (which, t_len)
    if key not in _NC_CACHE:
        _NC_CACHE[key] = build_kernel1(t_len) if which == 1 else build_kernel2(t_len)
    return _NC_CACHE[key]


TRACE = False  # set True (with an NTFF hook registered) to collect exec times
LAST_STATS = {}


def kernel(**inputs):
    from concourse import bass_utils

    gumbel = _gumbel_cpu()
    core_ids = list(range(NCORES))

    in_maps1, lens, maxlen = _host_pack_k1(inputs, gumbel)
    nc1 = _get_nc(1)
    res1 = bass_utils.run_bass_kernel_spmd(nc1, in_maps1, core_ids, trace=TRACE)
    ks_full = np.concatenate(
        [res1.results[c]["ks"].reshape(BPC, T) for c in range(NCORES)], axis=0)

    new_emb, new_lens, Ldyn = _host_compact(inputs, ks_full, lens, maxlen)
    in_maps2 = _host_pack_k2(inputs, new_emb, new_lens, Ldyn)
    nc2 = _get_nc(2)
    res2 = bass_utils.run_bass_kernel_spmd(nc2, in_maps2, core_ids, trace=TRACE)
    out = np.concatenate([res2.results[c]["out"][0] for c in range(NCORES)], axis=0)
    LAST_STATS["k1_ns"] = res1.exec_time_ns
    LAST_STATS["k2_ns"] = res2.exec_time_ns
    LAST_STATS["ks"] = ks_full
    LAST_STATS["new_lens"] = new_lens
    return out.astype(np.float32)



# revision 3
# speedup vs baseline: 1.3630x; 1.3630x over previous
"""Trainium2 Bass kernel for nn_CNN_RNN_88347477278730.

Pipeline (data-parallel over batch, 8 rows per core on 8 cores):
  kernel1 (device): chunked fp16 input projection (half hoisted, half
      interleaved into the early recurrence), then the 512-step
      select-policy GRUCell recurrence in full fp16 state with two 4-row
      batch streams for ILP; decisions batched into matmuls + is_gt at
      the end.
  host: compaction (gather kept tokens to the front), new_lens, Ldyn.
  kernel2 (device): compiled per dynamic sequence-length bucket t2
      (multiple of 32 >= max(new_lens)); chunked proj0, 2-layer GRU
      recurrence pipelined with a small lag, per-chunk proj1, Kim-CNN
      convs as shifted matmuls with compile-time pool windows, final
      linear.

All recurrence matmuls are gate-major (lhsT = weight tiles [K=128,
M=128], moving operand = h [K, batch]) so gate tensors land
partition-major where the elementwise engines are fast. The per-step
elementwise chain is 9 ops balanced across Vector/Scalar/GpSimd.
"""

import os
import subprocess
import sys
import tempfile

import numpy as np

# ---------------------------------------------------------------- constants
B, T, E, H, NF = 64, 512, 768, 256, 100
NCORES = 8
BPC = B // NCORES  # batch rows per core
KE = E // 128      # 6 K-tiles over the embedding dim
KH = H // 128      # 2 K-tiles over the hidden dim
GC = (3 * H) // 128  # 6 gate chunks (r: 0-1, z: 2-3, n: 4-5)

_F32 = None  # set lazily to mybir.dt.float32


# ------------------------------------------------------------- tile patch
def _apply_tile_patch():
    """This walrus build rejects >2 sem waits on one SP control instruction;
    split the TileContext tail drain into several drains of <=2 waits."""
    import concourse.tile as tile
    from concourse.vector_clock import ScopedClock, VectorClock

    if getattr(tile.TileContext, "_drain_split_patched", False):
        return

    def _patched(self, tick_clock, wait_clock):
        gc = tick_clock.global_clock
        n = len(gc)
        for start in range(0, n, 1):
            vec = [0] * n
            any_set = False
            for p in range(start, min(start + 1, n)):
                vec[p] = gc[p]
                any_set = any_set or vec[p] > 0
            if not any_set:
                continue
            d = self.nc.sync.drain()
            wait_clock.add_sem_waits(d.ins, ScopedClock({None: VectorClock(vec)}))
        self.nc.all_engine_barrier()
        assert self.sems is not None
        popped = self.nc._tile_sem_poison_stack.pop()
        assert popped is self._sem_poison
        self.nc.clear_and_free_semaphores(list(self.sems.allocated().values()))
        self.nc.all_engine_barrier()

    tile.TileContext._drain_and_barrier = _patched
    tile.TileContext._drain_split_patched = True


# ------------------------------------------------------------- gumbel (CPU)
def _gumbel_cpu():
    """jax.random.gumbel(key(42), (T-1, B, 2), f32) — computed in a CPU-jax
    subprocess so the accelerator backend is never involved (it must be
    bit-identical to the reference's CPU computation)."""
    path = os.path.join(tempfile.mkdtemp(), "gumbel.npy")
    code = (
        "import numpy as np, jax, jax.numpy as jnp\n"
        f"g = jax.random.gumbel(jax.random.key(42), ({T - 1}, {B}, 2), jnp.float32)\n"
        f"np.save({path!r}, np.asarray(g))\n"
    )
    env = dict(os.environ)
    env["TRN_TERMINAL_POOL_IPS"] = ""
    env["JAX_PLATFORMS"] = "cpu"
    extra = [p for p in sys.path if p and os.path.isdir(p)]
    env["PYTHONPATH"] = os.pathsep.join(extra)
    subprocess.run([sys.executable, "-c", code], env=env, check=True, capture_output=True)
    return np.load(path)


# ------------------------------------------------------------- host packing
def _pack_T(a2d):
    """[rows(=128*k), cols] -> [128, k, cols] weight-tile layout."""
    rows, cols = a2d.shape
    k = rows // 128
    return np.ascontiguousarray(a2d.reshape(k, 128, cols).transpose(1, 0, 2)).astype(np.float32)


def _pack_bias(b1d):
    """[128*k] -> [128, k]"""
    k = b1d.shape[0] // 128
    return np.ascontiguousarray(b1d.reshape(k, 128).T).astype(np.float32)


def _pack_embT(emb_rows, t_len=T):
    """[bpc, T, E] -> [KE, 128, bpc*T] (e-major tiles, free dims (b, t))."""
    bpc = emb_rows.shape[0]
    x = emb_rows.transpose(2, 0, 1).reshape(KE, 128, bpc * t_len)
    return np.ascontiguousarray(x).astype(np.float32)


def _pack_gru_weights(Wih, Whh, bih, bhh):
    """Returns (wihT, whhT, bias_proj, bhhn_rep) packings.

    bias_proj folds bih+bhh for the r,z chunks (added once at projection
    time); n chunks get bih only, with bhh_n applied per-step (it must be
    added to h@Whh_n *before* the r* multiply)."""
    wihT = _pack_T(np.ascontiguousarray(Wih.T))  # [128, KE or KH, 3H]
    whhT = _pack_T(np.ascontiguousarray(Whh.T))  # [128, KH, 3H]
    bias = np.empty(3 * H, np.float32)
    bias[: 2 * H] = bih[: 2 * H] + bhh[: 2 * H]
    bias[2 * H:] = bih[2 * H:]
    bias_proj = _pack_bias(bias)  # [128, GC]
    bhhn = _pack_bias(bhh[2 * H:])  # [128, KH]
    bhhn_rep = np.ascontiguousarray(
        np.broadcast_to(bhhn[:, :, None], (128, KH, BPC))
    ).astype(np.float32)
    return wihT, whhT, bias_proj, bhhn_rep


def _np_f16():
    from concourse import mybir

    return mybir.dt.np(mybir.dt.float16)


# ------------------------------------------------------------- bass builders
def _mk_nc():
    import concourse.bass as bass

    return bass.Bass("TRN2", target_bir_lowering=False, debug=False, num_devices=1)


def _split_excess_waits(nc, max_waits=1):
    """This walrus build can only encode ~2 sem waits per instruction
    (setupSyncWait 'Too many sync wait commands'). Hoist excess waits onto
    same-engine NoOps inserted just before the over-subscribed instruction;
    engine queues execute in order, so the wait semantics are identical."""
    from concourse import mybir

    nid = [0]
    for f in nc.m.functions:
        for bb in f.blocks:
            out = []
            changed = False
            for inst in bb.instructions:
                si = inst.sync_info
                lim = max_waits
                if si is not None and si.on_wait and len(si.on_wait) > lim:
                    waits = list(si.on_wait)
                    extra, keep = waits[:-lim], waits[-lim:]
                    for j in range(0, len(extra), max_waits):
                        nop = mybir.InstNoOp(
                            name=f"I-waitnop-{nid[0]}", ins=[], outs=[])
                        nid[0] += 1
                        nop.engine = inst.engine
                        nop.sync_info = mybir.SyncInfo(
                            on_wait=extra[j: j + max_waits], on_update=[])
                        nc.register_instruction(nop, overwrite=True)
                        out.append(nop)
                    inst.sync_info = mybir.SyncInfo(
                        on_wait=keep, on_update=list(si.on_update or []))
                    changed = True
                out.append(inst)
            if changed:
                bb.instructions = out
    return nc


def _proj_builder(nc, tc, misc_pool, dma_pool, big_ps, src_dram, wihT, biasc,
                  biasc_rep, gi, t_len, n_k, act, alu, f16, f32, tag):
    """Returns (emit_prefix, groups): chunked input projection.

    emit_prefix() emits the first t-half; `groups` is a list of closures,
    each emitting one (b, c) group of the second t-half (to be interleaved
    into the early recurrence steps). Copies alternate ACT (per-partition
    bias) / DVE (bias_rep tensor) to balance engines."""
    HALF = t_len // 2
    src_tiles = {}

    def _src(hb, b):
        key = (hb, b)
        if key not in src_tiles:
            s = dma_pool.tile([128, n_k, HALF], f16, tag=f"{tag}src")
            for k in range(n_k):
                nc.sync.dma_start(
                    s[:, k, :],
                    src_dram[k, :, b * t_len + hb * HALF: b * t_len + hb * HALF + HALF],
                )
            src_tiles[key] = s
        return src_tiles[key]

    def _group(hb, b, c):
        src = _src(hb, b)
        ps = big_ps.tile([128, HALF], f32, tag="bps")
        for k in range(n_k):
            nc.tensor.matmul(
                ps[:], wihT[:, k, c * 128: (c + 1) * 128], src[:, k, :],
                start=(k == 0), stop=(k == n_k - 1),
            )
        t0 = hb * HALF
        dst = gi[:, c, b, t0: t0 + HALF]
        if (b * GC + c) % 2 == 0:
            nc.scalar.activation(dst, ps[:], act.Identity, bias=biasc[:, c: c + 1])
        else:
            nc.vector.tensor_tensor(dst, ps[:], biasc_rep[:, c, :], alu.add)

    def emit_prefix():
        for b in range(BPC):
            for c in range(GC):
                _group(0, b, c)

    groups = [
        (lambda b=b, c=c: _group(1, b, c))
        for b in range(BPC) for c in range(GC)
    ]
    return emit_prefix, groups


def build_kernel1(t_len=T):
    """Select-policy kernel: fp16 everywhere (state, weights, projection);
    two 4-row batch streams interleaved for ILP; 9-op elementwise chain
    per stream-step; decisions batched into matmuls + is_gt at the end."""
    import concourse.tile as tile
    from concourse import mybir

    _apply_tile_patch()
    nc = _mk_nc()
    f32 = mybir.dt.float32
    f16 = mybir.dt.float16
    act = mybir.ActivationFunctionType
    alu = mybir.AluOpType
    SW = BPC // 2  # stream width (rows per stream)
    HALF = t_len // 2

    embT_d = nc.dram_tensor("embT", [KE, 128, BPC * t_len], f16, kind="ExternalInput").ap()
    wihcT_d = nc.dram_tensor("wihcT", [128, KE, 3 * H], f16, kind="ExternalInput").ap()
    biasc_d = nc.dram_tensor("biasc", [128, GC], f32, kind="ExternalInput").ap()
    biascr_d = nc.dram_tensor("biascr", [128, GC, HALF], f32, kind="ExternalInput").ap()
    whh16_d = nc.dram_tensor("whh16", [128, KH, 3 * H], f16, kind="ExternalInput").ap()
    bhhnc_d = nc.dram_tensor("bhhnc", [128, KH, BPC], f32, kind="ExternalInput").ap()
    wdiffT_d = nc.dram_tensor("wdiffT", [128, KH, 1], f16, kind="ExternalInput").ap()
    ncdiff_d = nc.dram_tensor("ncdiff", [1, BPC * t_len], f32, kind="ExternalInput").ap()
    ks_d = nc.dram_tensor("ks", [1, BPC * t_len], f32, kind="ExternalOutput").ap()

    with tile.TileContext(nc) as tc:
        from contextlib import ExitStack

        with ExitStack() as ctx:
            wpool = ctx.enter_context(tc.tile_pool(name="weights", bufs=1))
            gipool = ctx.enter_context(tc.tile_pool(name="gi", bufs=1))
            hpool = ctx.enter_context(tc.tile_pool(name="hist", bufs=1))
            dma_pool = ctx.enter_context(tc.tile_pool(name="dma", bufs=2))
            big_ps = ctx.enter_context(tc.tile_pool(name="bigps", bufs=2, space="PSUM"))
            rzA_ps = ctx.enter_context(tc.tile_pool(name="rzAps", bufs=1, space="PSUM"))
            nA_ps = ctx.enter_context(tc.tile_pool(name="nAps", bufs=1, space="PSUM"))
            sA_ps = ctx.enter_context(tc.tile_pool(name="sAps", bufs=1, space="PSUM"))
            rzB_ps = ctx.enter_context(tc.tile_pool(name="rzBps", bufs=1, space="PSUM"))
            nB_ps = ctx.enter_context(tc.tile_pool(name="nBps", bufs=1, space="PSUM"))
            sB_ps = ctx.enter_context(tc.tile_pool(name="sBps", bufs=1, space="PSUM"))
            sb_pool = ctx.enter_context(tc.tile_pool(name="gates", bufs=2))
            misc = ctx.enter_context(tc.tile_pool(name="misc", bufs=1))

            def _load(pool, dram, shape, tag, dt=f32):
                t_ = pool.tile(shape, dt, tag=tag)
                nc.sync.dma_start(t_[:], dram[:])
                return t_

            wihcT = _load(wpool, wihcT_d, [128, KE, 3 * H], "wihcT", f16)
            biasc = _load(wpool, biasc_d, [128, GC], "biasc")
            biascr = _load(wpool, biascr_d, [128, GC, HALF], "biascr")
            whh16 = _load(wpool, whh16_d, [128, KH, 3 * H], "whh16", f16)
            bhhnc_rep = _load(wpool, bhhnc_d, [128, KH, BPC], "bhhnc")
            wdiffT = _load(misc, wdiffT_d, [128, KH, 1], "wdiffT", f16)
            ncdiff = _load(misc, ncdiff_d, [1, BPC * t_len], "ncdiff")

            # gi layout: [128, GC, BPC, t] (t innermost: contiguous proj copies)
            gi = gipool.tile([128, GC, BPC, t_len], f16, tag="gi")
            emit_prefix, groups = _proj_builder(
                nc, tc, misc, dma_pool, big_ps, embT_d, wihcT, biasc, biascr,
                gi, t_len, KE, act, alu, f16, f32, "proj")
            emit_prefix()

            hist = hpool.tile([128, KH, BPC, t_len], f16, tag="hist")
            rzA = rzA_ps.tile([128, 4, SW], f32, tag="rzA")
            nA = nA_ps.tile([128, KH, SW], f32, tag="nA")
            sA = sA_ps.tile([128, 4, SW], f32, tag="sA")
            rzB = rzB_ps.tile([128, 4, SW], f32, tag="rzB")
            nB = nB_ps.tile([128, KH, SW], f32, tag="nB")
            sB = sB_ps.tile([128, 4, SW], f32, tag="sB")
            ps_map = {"a": (rzA, nA, sA), "b": (rzB, nB, sB)}

            def emit_step(t, b0, sfx):
                rz_ps, n_ps, s_ps = ps_map[sfx]
                bs = slice(b0, b0 + SW)
                rz = sb_pool.tile([128, 4, SW], f32, tag="rz" + sfx)
                tmp = sb_pool.tile([128, KH, SW], f32, tag="tmp" + sfx)
                if t > 0:
                    h_prev = hist[:, :, bs, t - 1]
                    for c in (0, 1, 2, 3, 4, 5):
                        dst = rz_ps[:, c, :] if c < 4 else n_ps[:, c - 4, :]
                        for k in range(KH):
                            nc.tensor.matmul(
                                dst, whh16[:, k, c * 128: (c + 1) * 128],
                                h_prev[:, k, :],
                                start=(k == 0), stop=(k == KH - 1),
                            )
                    nc.vector.tensor_tensor(s_ps[:], rz_ps[:], gi[:, 0:4, bs, t], alu.add)
                    nc.scalar.activation(rz[:], s_ps[:], act.Sigmoid)
                    nc.vector.tensor_tensor(tmp[:], n_ps[:], bhhnc_rep[:, :, bs], alu.add)
                else:
                    nc.scalar.activation(rz[:], gi[:, 0:4, bs, t], act.Sigmoid)
                    nc.vector.tensor_copy(tmp[:], bhhnc_rep[:, :, bs])
                nc.vector.tensor_tensor(tmp[:], tmp[:], rz[:, 0:KH, :], alu.mult)
                nc.gpsimd.tensor_tensor(tmp[:], tmp[:], gi[:, 4:GC, bs, t], alu.add)
                nn_ = sb_pool.tile([128, KH, SW], f32, tag="nn" + sfx)
                nc.scalar.activation(nn_[:], tmp[:], act.Tanh)
                d = sb_pool.tile([128, KH, SW], f32, tag="dd" + sfx)
                if t > 0:
                    nc.gpsimd.tensor_tensor(d[:], hist[:, :, bs, t - 1], nn_[:], alu.subtract)
                else:
                    nc.gpsimd.tensor_scalar(d[:], nn_[:], -1.0, None, alu.mult)
                nc.vector.tensor_tensor(d[:], rz[:, 2:4, :], d[:], alu.mult)
                nc.gpsimd.tensor_tensor(hist[:, :, bs, t], nn_[:], d[:], alu.add)

            gidx = 0
            for t in range(t_len):
                emit_step(t, 0, "a")
                emit_step(t, SW, "b")
                if t % 2 == 1 and gidx < len(groups):
                    groups[gidx]()
                    gidx += 1
            while gidx < len(groups):
                groups[gidx]()
                gidx += 1

            # ---- batched decisions: ks[b,t] = (h_t . wdiff > ncdiff) ----
            ks_sb = misc.tile([1, BPC * t_len], f32, tag="kssb")
            for b in range(BPC):
                dps = big_ps.tile([1, t_len], f32, tag="bps")
                for k in range(KH):
                    nc.tensor.matmul(
                        dps[:], wdiffT[:, k, :], hist[:, k, b, :],
                        start=(k == 0), stop=(k == KH - 1),
                    )
                nc.vector.tensor_tensor(
                    ks_sb[0:1, b * t_len: (b + 1) * t_len], dps[:],
                    ncdiff[0:1, b * t_len: (b + 1) * t_len], alu.is_gt,
                )
            nc.sync.dma_start(ks_d[:], ks_sb[:])

    return _split_excess_waits(nc)


def build_kernel2(t2, kf3, kf4, kf5):
    """GRU0/GRU1 + convs + pooling + final linear at dynamic length t2.

    L0 and L1 recurrences pipelined with a LAG-step lag; gi1 computed
    per-D-chunk from the o1 history; conv max-pool windows (kf*) are
    compile-time constants so no time-mask tensor is needed."""
    import concourse.tile as tile
    from concourse import mybir

    _apply_tile_patch()
    nc = _mk_nc()
    f32 = mybir.dt.float32
    f16 = mybir.dt.float16
    act = mybir.ActivationFunctionType
    alu = mybir.AluOpType
    D = 32
    LAG = D + 8
    NCH = t2 // D
    HALF = t2 // 2

    nembT_d = nc.dram_tensor("nembT", [KE, 128, BPC * t2], f16, kind="ExternalInput").ap()
    wih0T_d = nc.dram_tensor("wih0T", [128, KE, 3 * H], f16, kind="ExternalInput").ap()
    whh0T_d = nc.dram_tensor("whh0T", [128, KH, 3 * H], f16, kind="ExternalInput").ap()
    bias0_d = nc.dram_tensor("bias0", [128, GC], f32, kind="ExternalInput").ap()
    bias0r_d = nc.dram_tensor("bias0r", [128, GC, HALF], f32, kind="ExternalInput").ap()
    bhhn0_d = nc.dram_tensor("bhhn0", [128, KH, BPC], f32, kind="ExternalInput").ap()
    wih1T_d = nc.dram_tensor("wih1T", [128, KH, 3 * H], f16, kind="ExternalInput").ap()
    whh1T_d = nc.dram_tensor("whh1T", [128, KH, 3 * H], f16, kind="ExternalInput").ap()
    bias1_d = nc.dram_tensor("bias1", [128, GC], f32, kind="ExternalInput").ap()
    bias1r_d = nc.dram_tensor("bias1r", [128, GC, D], f32, kind="ExternalInput").ap()
    bhhn1_d = nc.dram_tensor("bhhn1", [128, KH, BPC], f32, kind="ExternalInput").ap()
    vt_d = nc.dram_tensor("vt", [1, BPC * t2], f16, kind="ExternalInput").ap()
    cw_d = nc.dram_tensor("cw", [128, 12, KH, NF], f16, kind="ExternalInput").ap()
    cb_d = nc.dram_tensor("cb", [NF, 3], f32, kind="ExternalInput").ap()
    woutT_d = nc.dram_tensor("woutT", [NF, 3], f32, kind="ExternalInput").ap()
    bout_d = nc.dram_tensor("bout", [1, 1], f32, kind="ExternalInput").ap()
    out_d = nc.dram_tensor("out", [1, BPC], f32, kind="ExternalOutput").ap()

    FS = (3, 4, 5)
    KFS = (kf3, kf4, kf5)

    with tile.TileContext(nc) as tc:
        from contextlib import ExitStack

        with ExitStack() as ctx:
            wpool = ctx.enter_context(tc.tile_pool(name="weights", bufs=1))
            gipool = ctx.enter_context(tc.tile_pool(name="gi", bufs=1))
            opool = ctx.enter_context(tc.tile_pool(name="obuf", bufs=1))
            dma_pool = ctx.enter_context(tc.tile_pool(name="dma", bufs=2))
            big_ps = ctx.enter_context(tc.tile_pool(name="bigps", bufs=2, space="PSUM"))
            rz0_ps = ctx.enter_context(tc.tile_pool(name="rz0ps", bufs=1, space="PSUM"))
            n0_ps = ctx.enter_context(tc.tile_pool(name="n0ps", bufs=1, space="PSUM"))
            s0_ps = ctx.enter_context(tc.tile_pool(name="s0ps", bufs=1, space="PSUM"))
            rz1_ps = ctx.enter_context(tc.tile_pool(name="rz1ps", bufs=1, space="PSUM"))
            n1_ps = ctx.enter_context(tc.tile_pool(name="n1ps", bufs=1, space="PSUM"))
            s1_ps = ctx.enter_context(tc.tile_pool(name="s1ps", bufs=1, space="PSUM"))
            sb_pool = ctx.enter_context(tc.tile_pool(name="gates", bufs=2))
            misc = ctx.enter_context(tc.tile_pool(name="misc", bufs=1))

            def _load(pool, dram, shape, tag, dt=f32):
                t_ = pool.tile(shape, dt, tag=tag)
                nc.sync.dma_start(t_[:], dram[:])
                return t_

            wih0T = _load(wpool, wih0T_d, [128, KE, 3 * H], "wih0", f16)
            whh0T = _load(wpool, whh0T_d, [128, KH, 3 * H], "whh0", f16)
            bias0 = _load(wpool, bias0_d, [128, GC], "bias0")
            bias0r = _load(wpool, bias0r_d, [128, GC, HALF], "bias0r")
            bhhn0 = _load(wpool, bhhn0_d, [128, KH, BPC], "bhhn0")
            wih1T = _load(wpool, wih1T_d, [128, KH, 3 * H], "wih1", f16)
            whh1T = _load(wpool, whh1T_d, [128, KH, 3 * H], "whh1", f16)
            bias1 = _load(wpool, bias1_d, [128, GC], "bias1")
            bias1r = _load(wpool, bias1r_d, [128, GC, D], "bias1r")
            bhhn1 = _load(wpool, bhhn1_d, [128, KH, BPC], "bhhn1")
            cw = _load(wpool, cw_d, [128, 12, KH, NF], "cw", f16)
            cb = _load(misc, cb_d, [NF, 3], "cb")
            woutT = _load(misc, woutT_d, [NF, 3], "woutT")
            bout = _load(misc, bout_d, [1, 1], "bout")
            vt = _load(misc, vt_d, [1, BPC * t2], "vt", f16)

            gi0 = gipool.tile([128, GC, BPC, t2], f16, tag="gi0")
            gi1 = gipool.tile([128, GC, BPC, t2], f16, tag="gi1")
            o1 = opool.tile([128, KH, BPC, t2], f16, tag="o1")
            o2 = opool.tile([128, KH, BPC, t2], f16, tag="o2")

            emit_prefix, groups = _proj_builder(
                nc, tc, misc, dma_pool, big_ps, nembT_d, wih0T, bias0, bias0r,
                gi0, t2, KE, act, alu, f16, f32, "proj0")
            emit_prefix()

            rz0 = rz0_ps.tile([128, 4, BPC], f32, tag="rz0")
            n0 = n0_ps.tile([128, KH, BPC], f32, tag="n0")
            s0 = s0_ps.tile([128, 4, BPC], f32, tag="s0")
            rz1 = rz1_ps.tile([128, 4, BPC], f32, tag="rz1")
            n1 = n1_ps.tile([128, KH, BPC], f32, tag="n1")
            s1 = s1_ps.tile([128, 4, BPC], f32, tag="s1")

            def emit_step(t, gi, whh, bhhn, hist, rz_ps, n_ps, s_ps, sfx):
                rz = sb_pool.tile([128, 4, BPC], f32, tag="rz" + sfx)
                tmp = sb_pool.tile([128, KH, BPC], f32, tag="tmp" + sfx)
                if t > 0:
                    h_prev = hist[:, :, :, t - 1]
                    for c in (0, 1, 2, 3, 4, 5):
                        dst = rz_ps[:, c, :] if c < 4 else n_ps[:, c - 4, :]
                        for k in range(KH):
                            nc.tensor.matmul(
                                dst, whh[:, k, c * 128: (c + 1) * 128],
                                h_prev[:, k, :],
                                start=(k == 0), stop=(k == KH - 1),
                            )
                    nc.vector.tensor_tensor(s_ps[:], rz_ps[:], gi[:, 0:4, :, t], alu.add)
                    nc.scalar.activation(rz[:], s_ps[:], act.Sigmoid)
                    nc.vector.tensor_tensor(tmp[:], n_ps[:], bhhn[:], alu.add)
                else:
                    nc.scalar.activation(rz[:], gi[:, 0:4, :, t], act.Sigmoid)
                    nc.vector.tensor_copy(tmp[:], bhhn[:])
                nc.vector.tensor_tensor(tmp[:], tmp[:], rz[:, 0:KH, :], alu.mult)
                nc.gpsimd.tensor_tensor(tmp[:], tmp[:], gi[:, 4:GC, :, t], alu.add)
                nn_ = sb_pool.tile([128, KH, BPC], f32, tag="nn" + sfx)
                nc.scalar.activation(nn_[:], tmp[:], act.Tanh)
                d = sb_pool.tile([128, KH, BPC], f32, tag="dd" + sfx)
                if t > 0:
                    nc.gpsimd.tensor_tensor(d[:], hist[:, :, :, t - 1], nn_[:], alu.subtract)
                else:
                    nc.gpsimd.tensor_scalar(d[:], nn_[:], -1.0, None, alu.mult)
                if sfx == "0":
                    nc.vector.tensor_tensor(d[:], rz[:, 2:4, :], d[:], alu.mult)
                else:
                    nc.gpsimd.tensor_tensor(d[:], rz[:, 2:4, :], d[:], alu.mult)
                nc.gpsimd.tensor_tensor(hist[:, :, :, t], nn_[:], d[:], alu.add)

            def emit_proj1_chunk(ci):
                t0, t1 = ci * D, (ci + 1) * D
                for b in range(BPC):
                    for c in range(GC):
                        ps = big_ps.tile([128, D], f32, tag="bps")
                        for k in range(KH):
                            nc.tensor.matmul(
                                ps[:], wih1T[:, k, c * 128: (c + 1) * 128],
                                o1[:, k, b, t0:t1],
                                start=(k == 0), stop=(k == KH - 1),
                            )
                        dst = gi1[:, c, b, t0:t1]
                        if (b * GC + c) % 2 == 0:
                            nc.scalar.activation(
                                dst, ps[:], act.Identity, bias=bias1[:, c: c + 1])
                        else:
                            nc.vector.tensor_tensor(dst, ps[:], bias1r[:, c, :], alu.add)

            gidx = 0
            for t in range(t2 + LAG):
                if t < t2:
                    emit_step(t, gi0, whh0T, bhhn0, o1, rz0, n0, s0, "0")
                    if t % 2 == 1 and gidx < len(groups):
                        groups[gidx]()
                        gidx += 1
                if t >= LAG:
                    emit_step(t - LAG, gi1, whh1T, bhhn1, o2, rz1, n1, s1, "1")
                if t < t2 and t % D == D - 1:
                    emit_proj1_chunk(t // D)

            # ---- zero o2 past new_lens: o2 *= vt ----
            # partition-broadcast vt via a K=1 ones-matmul (PE outer product)
            ones_sb = misc.tile([1, 128], f16, tag="ones")
            nc.vector.memset(ones_sb[:], 1.0)
            for b in range(BPC):
                vtb = big_ps.tile([128, t2], f32, tag="bps")
                nc.tensor.matmul(
                    vtb[:], ones_sb[:], vt[0:1, b * t2: (b + 1) * t2],
                    start=True, stop=True,
                )
                for k in range(KH):
                    nc.vector.tensor_tensor(
                        o2[:, k, b, :], o2[:, k, b, :], vtb[:], alu.mult
                    )

            # ---- convs + relu + max-pool over compile-time window ----
            pooled = misc.tile([NF, 3, BPC], f32, tag="pooled")
            for b in range(BPC):
                for fi, fs in enumerate(FS):
                    nw = t2 - fs + 1
                    kf = KFS[fi]
                    ps = big_ps.tile([NF, t2], f32, tag="bps")
                    m0 = sum(FS[:fi])  # flat (fs,dt) base index
                    first = True
                    for dt_ in range(fs):
                        for k in range(KH):
                            nc.tensor.matmul(
                                ps[:, :nw],
                                cw[:, m0 + dt_, k, :],
                                o2[:, k, b, dt_: dt_ + nw],
                                start=first,
                                stop=(dt_ == fs - 1 and k == KH - 1),
                            )
                            first = False
                    crelu = sb_pool.tile([NF, t2], f32, tag="crelu")
                    nc.scalar.activation(
                        crelu[:, :kf], ps[:, :kf], act.Relu, bias=cb[:, fi: fi + 1]
                    )
                    nc.vector.tensor_reduce(
                        pooled[:, fi, b: b + 1], crelu[:, :kf],
                        mybir.AxisListType.X, alu.max,
                    )

            # ---- final linear ----
            fps = big_ps.tile([1, BPC], f32, tag="bps")
            for fi in range(3):
                nc.tensor.matmul(
                    fps[:], woutT[:, fi: fi + 1], pooled[:, fi, :],
                    start=(fi == 0), stop=(fi == 2),
                )
            out_sb = misc.tile([1, BPC], f32, tag="outsb")
            nc.scalar.activation(out_sb[:], fps[:], act.Identity, bias=bout[0:1, 0:1])
            nc.sync.dma_start(out_d[:], out_sb[:])

    return _split_excess_waits(nc)


# ------------------------------------------------------------- host orchestration
def _host_pack_k1(inputs, gumbel, t_len=T):
    f16 = _np_f16()
    emb = np.asarray(inputs["embedded"], np.float32)
    mask = np.asarray(inputs["mask"])
    lens = mask.sum(1)
    maxlen = int(lens.max())

    wihcT, whhT, biasc, bhhnc = _pack_gru_weights(
        inputs["Wih_c"], inputs["Whh_c"], inputs["bih_c"], inputs["bhh_c"])
    wdiff = (inputs["Wsel"][1] - inputs["Wsel"][0]).astype(np.float32)
    wdiffT = np.ascontiguousarray(wdiff.reshape(KH, 128).T[:, :, None])
    bdiff = float(inputs["bsel"][1] - inputs["bsel"][0])

    # ncdiff[b, t]: k_t = (h.wdiff > ncdiff); forced off when t >= maxlen-1
    ncdiff = np.full((B, t_len), 1.0e30, np.float32)
    upto = min(maxlen - 1, t_len)
    for t in range(1, upto):
        ncdiff[:, t] = -(bdiff + gumbel[t - 1, :, 1] - gumbel[t - 1, :, 0])

    biascr = np.ascontiguousarray(
        np.broadcast_to(biasc[:, :, None], (128, GC, t_len // 2))).astype(np.float32)

    in_maps = []
    for c in range(NCORES):
        rows = slice(c * BPC, (c + 1) * BPC)
        in_maps.append({
            "embT": _pack_embT(emb[rows, :t_len], t_len).astype(f16),
            "wihcT": wihcT.astype(f16),
            "biasc": biasc,
            "biascr": biascr,
            "whh16": whhT.astype(f16),
            "bhhnc": bhhnc,
            "wdiffT": wdiffT.astype(f16),
            "ncdiff": np.ascontiguousarray(
                ncdiff[rows].reshape(1, BPC * t_len)),
        })
    return in_maps, lens, maxlen


def _host_compact(inputs, ks_full, lens, maxlen, t_len=T):
    """ks_full: [B, t_len] decision bits (row t=0 ignored; selected[:,0]=1)."""
    emb = np.asarray(inputs["embedded"], np.float32)
    selected = np.zeros((B, t_len), np.int64)
    selected[:, 0] = 1
    selected[:, 1:] = ks_full[:, 1:]
    pos = np.arange(t_len)
    sel_valid = np.where(pos[None, :] < (lens - 1)[:, None], selected, 0)
    new_mask = np.where(pos[None, :] == (lens - 1)[:, None], 1, sel_valid)
    new_lens = new_mask.sum(1)
    Ldyn = max(int(new_lens.max()), 7)

    t2 = max(-(-Ldyn // 64) * 64, 64)
    new_emb = np.zeros((B, t2, E), np.float32)
    for b in range(B):
        idx = np.nonzero(new_mask[b])[0]
        new_emb[b, : len(idx)] = emb[b, idx]
    return new_emb, new_lens, Ldyn, t2


def _host_pack_k2(inputs, new_emb, new_lens, Ldyn, t2):
    f16 = _np_f16()
    wih0T, whh0T, bias0, bhhn0 = _pack_gru_weights(
        inputs["Wih0"], inputs["Whh0"], inputs["bih0"], inputs["bhh0"])
    wih1T, whh1T, bias1, bhhn1 = _pack_gru_weights(
        inputs["Wih1"], inputs["Whh1"], inputs["bih1"], inputs["bhh1"])

    FS = (3, 4, 5)
    cw = np.zeros((128, 12, KH, NF), np.float32)
    cb = np.zeros((NF, 3), np.float32)
    m = 0
    for fi, fs in enumerate(FS):
        w = np.asarray(inputs[f"conv_w{fs}"], np.float32)  # [NF,1,fs,H]
        cb[:, fi] = np.asarray(inputs[f"conv_b{fs}"], np.float32)
        for dt_ in range(fs):
            wt = w[:, 0, dt_, :].T  # [H, NF]
            cw[:, m, :, :] = wt.reshape(KH, 128, NF).transpose(1, 0, 2)
            m += 1

    woutT = np.ascontiguousarray(
        np.asarray(inputs["Wout"], np.float32)[0].reshape(3, NF).T)
    bout = np.asarray(inputs["bout"], np.float32).reshape(1, 1)

    vt_full = (np.arange(t2)[None, :] < new_lens[:, None]).astype(np.float32)

    bias0r = np.ascontiguousarray(
        np.broadcast_to(bias0[:, :, None], (128, GC, t2 // 2))).astype(np.float32)
    bias1r = np.ascontiguousarray(
        np.broadcast_to(bias1[:, :, None], (128, GC, 32))).astype(np.float32)

    in_maps = []
    for c in range(NCORES):
        rows = slice(c * BPC, (c + 1) * BPC)
        in_maps.append({
            "nembT": _pack_embT(new_emb[rows], t2).astype(f16),
            "wih0T": wih0T.astype(f16), "whh0T": whh0T.astype(f16),
            "bias0": bias0, "bias0r": bias0r, "bhhn0": bhhn0,
            "wih1T": wih1T.astype(f16), "whh1T": whh1T.astype(f16),
            "bias1": bias1, "bias1r": bias1r, "bhhn1": bhhn1,
            "vt": np.ascontiguousarray(
                vt_full[rows].reshape(1, BPC * t2)).astype(f16),
            "cw": cw.astype(f16), "cb": cb,
            "woutT": woutT, "bout": bout,
        })
    return in_maps


_NC_CACHE = {}


def _get_nc1(t_len=T):
    key = (1, t_len)
    if key not in _NC_CACHE:
        _NC_CACHE[key] = build_kernel1(t_len)
    return _NC_CACHE[key]


def _get_nc2(t2, kfs):
    key = (2, t2, kfs)
    if key not in _NC_CACHE:
        _NC_CACHE[key] = build_kernel2(t2, *kfs)
    return _NC_CACHE[key]


TRACE = False  # set True (with an NTFF hook registered) to collect exec times
LAST_STATS = {}


def kernel(**inputs):
    from concourse import bass_utils

    gumbel = _gumbel_cpu()
    core_ids = list(range(NCORES))

    in_maps1, lens, maxlen = _host_pack_k1(inputs, gumbel)
    nc1 = _get_nc1()
    res1 = bass_utils.run_bass_kernel_spmd(nc1, in_maps1, core_ids, trace=TRACE)
    ks_full = np.concatenate(
        [res1.results[c]["ks"].reshape(BPC, T) for c in range(NCORES)], axis=0)

    new_emb, new_lens, Ldyn, t2 = _host_compact(inputs, ks_full, lens, maxlen)
    kfs = tuple(min(Ldyn - fs + 1, t2 - fs + 1) for fs in (3, 4, 5))
    in_maps2 = _host_pack_k2(inputs, new_emb, new_lens, Ldyn, t2)
    nc2 = _get_nc2(t2, kfs)
    res2 = bass_utils.run_bass_kernel_spmd(nc2, in_maps2, core_ids, trace=TRACE)
    out = np.concatenate([res2.results[c]["out"][0] for c in range(NCORES)], axis=0)
    LAST_STATS["k1_ns"] = res1.exec_time_ns
    LAST_STATS["k2_ns"] = res2.exec_time_ns
    LAST_STATS["ks"] = ks_full
    LAST_STATS["new_lens"] = new_lens
    return out.astype(np.float32)


# revision 5
# speedup vs baseline: 1.4553x; 1.0678x over previous
"""Trainium2 Bass kernel for nn_CNN_RNN_88347477278730.

Pipeline (data-parallel over batch, 8 rows per core on 8 cores):
  kernel1 (device): chunked fp16 input projection (half hoisted, half
      interleaved into the early recurrence), then the 512-step
      select-policy GRUCell recurrence in full fp16 state with two 4-row
      batch streams for ILP; decisions batched into matmuls + is_gt at
      the end.
  host: compaction (gather kept tokens to the front), new_lens, Ldyn.
  kernel2 (device): compiled per dynamic sequence-length bucket t2
      (multiple of 32 >= max(new_lens)); chunked proj0, 2-layer GRU
      recurrence pipelined with a small lag, per-chunk proj1, Kim-CNN
      convs as shifted matmuls with compile-time pool windows, final
      linear.

All recurrence matmuls are gate-major (lhsT = weight tiles [K=128,
M=128], moving operand = h [K, batch]) so gate tensors land
partition-major where the elementwise engines are fast. The per-step
elementwise chain is 9 ops balanced across Vector/Scalar/GpSimd.
"""

import os
import subprocess
import sys
import tempfile

import numpy as np

# ---------------------------------------------------------------- constants
B, T, E, H, NF = 64, 512, 768, 256, 100
NCORES = 8
BPC = B // NCORES  # batch rows per core
KE = E // 128      # 6 K-tiles over the embedding dim
KH = H // 128      # 2 K-tiles over the hidden dim
GC = (3 * H) // 128  # 6 gate chunks (r: 0-1, z: 2-3, n: 4-5)

_F32 = None  # set lazily to mybir.dt.float32


# ------------------------------------------------------------- tile patch
def _apply_tile_patch():
    """This walrus build rejects >2 sem waits on one SP control instruction;
    split the TileContext tail drain into several drains of <=2 waits."""
    import concourse.tile as tile
    from concourse.vector_clock import ScopedClock, VectorClock

    if getattr(tile.TileContext, "_drain_split_patched", False):
        return

    def _patched(self, tick_clock, wait_clock):
        gc = tick_clock.global_clock
        n = len(gc)
        for start in range(0, n, 1):
            vec = [0] * n
            any_set = False
            for p in range(start, min(start + 1, n)):
                vec[p] = gc[p]
                any_set = any_set or vec[p] > 0
            if not any_set:
                continue
            d = self.nc.sync.drain()
            wait_clock.add_sem_waits(d.ins, ScopedClock({None: VectorClock(vec)}))
        self.nc.all_engine_barrier()
        assert self.sems is not None
        popped = self.nc._tile_sem_poison_stack.pop()
        assert popped is self._sem_poison
        self.nc.clear_and_free_semaphores(list(self.sems.allocated().values()))
        self.nc.all_engine_barrier()

    tile.TileContext._drain_and_barrier = _patched
    tile.TileContext._drain_split_patched = True


# ------------------------------------------------------------- gumbel (CPU)
def _gumbel_cpu():
    """jax.random.gumbel(key(42), (T-1, B, 2), f32) — computed in a CPU-jax
    subprocess so the accelerator backend is never involved (it must be
    bit-identical to the reference's CPU computation)."""
    path = os.path.join(tempfile.mkdtemp(), "gumbel.npy")
    code = (
        "import numpy as np, jax, jax.numpy as jnp\n"
        f"g = jax.random.gumbel(jax.random.key(42), ({T - 1}, {B}, 2), jnp.float32)\n"
        f"np.save({path!r}, np.asarray(g))\n"
    )
    env = dict(os.environ)
    env["TRN_TERMINAL_POOL_IPS"] = ""
    env["JAX_PLATFORMS"] = "cpu"
    extra = [p for p in sys.path if p and os.path.isdir(p)]
    env["PYTHONPATH"] = os.pathsep.join(extra)
    subprocess.run([sys.executable, "-c", code], env=env, check=True, capture_output=True)
    return np.load(path)


# ------------------------------------------------------------- host packing
def _pack_T(a2d):
    """[rows(=128*k), cols] -> [128, k, cols] weight-tile layout."""
    rows, cols = a2d.shape
    k = rows // 128
    return np.ascontiguousarray(a2d.reshape(k, 128, cols).transpose(1, 0, 2)).astype(np.float32)


def _pack_bias(b1d):
    """[128*k] -> [128, k]"""
    k = b1d.shape[0] // 128
    return np.ascontiguousarray(b1d.reshape(k, 128).T).astype(np.float32)


def _pack_embT(emb_rows, t_len=T):
    """[bpc, T, E] -> [KE, 128, bpc*T] (e-major tiles, free dims (b, t))."""
    bpc = emb_rows.shape[0]
    x = emb_rows.transpose(2, 0, 1).reshape(KE, 128, bpc * t_len)
    return np.ascontiguousarray(x).astype(np.float32)


def _pack_gru_weights(Wih, Whh, bih, bhh):
    """Returns (wihT, whhT, bias_proj, bhhn_rep) packings.

    bias_proj folds bih+bhh for the r,z chunks (added once at projection
    time); n chunks get bih only, with bhh_n applied per-step (it must be
    added to h@Whh_n *before* the r* multiply)."""
    wihT = _pack_T(np.ascontiguousarray(Wih.T))  # [128, KE or KH, 3H]
    whhT = _pack_T(np.ascontiguousarray(Whh.T))  # [128, KH, 3H]
    bias = np.empty(3 * H, np.float32)
    bias[: 2 * H] = bih[: 2 * H] + bhh[: 2 * H]
    bias[2 * H:] = bih[2 * H:]
    bias_proj = _pack_bias(bias)  # [128, GC]
    bhhn = _pack_bias(bhh[2 * H:])  # [128, KH]
    bhhn_rep = np.ascontiguousarray(
        np.broadcast_to(bhhn[:, :, None], (128, KH, BPC))
    ).astype(np.float32)
    return wihT, whhT, bias_proj, bhhn_rep


def _np_f16():
    from concourse import mybir

    return mybir.dt.np(mybir.dt.float16)


# ------------------------------------------------------------- bass builders
def _mk_nc():
    import concourse.bass as bass

    return bass.Bass("TRN2", target_bir_lowering=False, debug=False, num_devices=1)


def _split_excess_waits(nc, max_waits=1):
    """This walrus build can only encode ~2 sem waits per instruction
    (setupSyncWait 'Too many sync wait commands'). Hoist excess waits onto
    same-engine NoOps inserted just before the over-subscribed instruction;
    engine queues execute in order, so the wait semantics are identical."""
    from concourse import mybir

    nid = [0]
    for f in nc.m.functions:
        for bb in f.blocks:
            out = []
            changed = False
            for inst in bb.instructions:
                si = inst.sync_info
                lim = max_waits
                if si is not None and si.on_wait and len(si.on_wait) > lim:
                    waits = list(si.on_wait)
                    extra, keep = waits[:-lim], waits[-lim:]
                    for j in range(0, len(extra), max_waits):
                        nop = mybir.InstNoOp(
                            name=f"I-waitnop-{nid[0]}", ins=[], outs=[])
                        nid[0] += 1
                        nop.engine = inst.engine
                        nop.sync_info = mybir.SyncInfo(
                            on_wait=extra[j: j + max_waits], on_update=[])
                        nc.register_instruction(nop, overwrite=True)
                        out.append(nop)
                    inst.sync_info = mybir.SyncInfo(
                        on_wait=keep, on_update=list(si.on_update or []))
                    changed = True
                out.append(inst)
            if changed:
                bb.instructions = out
    return nc


def _proj_builder(nc, tc, misc_pool, dma_pool, big_ps, src_dram, wihT, biasc,
                  biasc_rep, gi, t_len, n_k, act, alu, f16, f32, tag):
    """Returns (emit_prefix, groups): chunked input projection.

    emit_prefix() emits the first t-half; `groups` is a list of closures,
    each emitting one (b, c) group of the second t-half (to be interleaved
    into the early recurrence steps). Copies alternate ACT (per-partition
    bias) / DVE (bias_rep tensor) to balance engines."""
    HALF = t_len // 2
    src_tiles = {}

    def _src(hb, b):
        key = (hb, b)
        if key not in src_tiles:
            s = dma_pool.tile([128, n_k, HALF], f16, tag=f"{tag}src")
            for k in range(n_k):
                nc.sync.dma_start(
                    s[:, k, :],
                    src_dram[k, :, b * t_len + hb * HALF: b * t_len + hb * HALF + HALF],
                )
            src_tiles[key] = s
        return src_tiles[key]

    def _group(hb, b, c):
        src = _src(hb, b)
        ps = big_ps.tile([128, HALF], f32, tag="bps")
        for k in range(n_k):
            nc.tensor.matmul(
                ps[:], wihT[:, k, c * 128: (c + 1) * 128], src[:, k, :],
                start=(k == 0), stop=(k == n_k - 1),
            )
        t0 = hb * HALF
        dst = gi[:, c, b, t0: t0 + HALF]
        if (b * GC + c) % 2 == 0:
            nc.scalar.activation(dst, ps[:], act.Identity, bias=biasc[:, c: c + 1])
        else:
            nc.vector.tensor_tensor(dst, ps[:], biasc_rep[:, c, :], alu.add)

    def emit_prefix():
        for b in range(BPC):
            for c in range(GC):
                _group(0, b, c)

    groups = [
        (lambda b=b, c=c: _group(1, b, c))
        for b in range(BPC) for c in range(GC)
    ]
    return emit_prefix, groups


def build_kernel1(t_len=T):
    """Select-policy kernel: fp16 everywhere; ONE merged 8-row stream (the
    per-step serial chain latency is the period — extra streams only add
    engine-queue coupling); gi_rz and bhh_n preloaded into PSUM off-chain so
    the gate matmuls accumulate straight onto them (start=False after a
    has_written-priming dummy matmul); 7-op chain; decisions batched at
    the end."""
    import concourse.tile as tile
    from concourse import mybir

    _apply_tile_patch()
    nc = _mk_nc()
    f32 = mybir.dt.float32
    f16 = mybir.dt.float16
    act = mybir.ActivationFunctionType
    alu = mybir.AluOpType
    HALF = t_len // 2

    embT_d = nc.dram_tensor("embT", [KE, 128, BPC * t_len], f16, kind="ExternalInput").ap()
    wihcT_d = nc.dram_tensor("wihcT", [128, KE, 3 * H], f16, kind="ExternalInput").ap()
    biasc_d = nc.dram_tensor("biasc", [128, GC], f32, kind="ExternalInput").ap()
    biascr_d = nc.dram_tensor("biascr", [128, GC, HALF], f32, kind="ExternalInput").ap()
    whh16_d = nc.dram_tensor("whh16", [128, KH, 3 * H], f16, kind="ExternalInput").ap()
    bhhnc_d = nc.dram_tensor("bhhnc", [128, KH, BPC], f32, kind="ExternalInput").ap()
    wdiffT_d = nc.dram_tensor("wdiffT", [128, KH, 1], f16, kind="ExternalInput").ap()
    ncdiff_d = nc.dram_tensor("ncdiff", [1, BPC * t_len], f32, kind="ExternalInput").ap()
    ks_d = nc.dram_tensor("ks", [1, BPC * t_len], f32, kind="ExternalOutput").ap()

    with tile.TileContext(nc) as tc:
        from contextlib import ExitStack

        with ExitStack() as ctx:
            wpool = ctx.enter_context(tc.tile_pool(name="weights", bufs=1))
            gipool = ctx.enter_context(tc.tile_pool(name="gi", bufs=1))
            hpool = ctx.enter_context(tc.tile_pool(name="hist", bufs=1))
            dma_pool = ctx.enter_context(tc.tile_pool(name="dma", bufs=2))
            big_ps = ctx.enter_context(tc.tile_pool(name="bigps", bufs=2, space="PSUM"))
            rz_psp = ctx.enter_context(tc.tile_pool(name="rzps", bufs=1, space="PSUM"))
            n_psp = ctx.enter_context(tc.tile_pool(name="nps", bufs=1, space="PSUM"))
            sb_pool = ctx.enter_context(tc.tile_pool(name="gates", bufs=2))
            misc = ctx.enter_context(tc.tile_pool(name="misc", bufs=1))

            def _load(pool, dram, shape, tag, dt=f32):
                t_ = pool.tile(shape, dt, tag=tag)
                nc.sync.dma_start(t_[:], dram[:])
                return t_

            wihcT = _load(wpool, wihcT_d, [128, KE, 3 * H], "wihcT", f16)
            biasc = _load(wpool, biasc_d, [128, GC], "biasc")
            biascr = _load(wpool, biascr_d, [128, GC, HALF], "biascr")
            whh16 = _load(wpool, whh16_d, [128, KH, 3 * H], "whh16", f16)
            bhhnc_rep = _load(wpool, bhhnc_d, [128, KH, BPC], "bhhnc")
            wdiffT = _load(misc, wdiffT_d, [128, KH, 1], "wdiffT", f16)
            ncdiff = _load(misc, ncdiff_d, [1, BPC * t_len], "ncdiff")
            zs = misc.tile([1, 128], f16, tag="zs")
            zx = misc.tile([1, 4 * BPC], f16, tag="zx")
            nc.vector.memset(zs[:], 0.0)
            nc.vector.memset(zx[:], 0.0)

            # gi layout: [128, GC, BPC, t] (t innermost: contiguous proj copies)
            gi = gipool.tile([128, GC, BPC, t_len], f16, tag="gi")
            emit_prefix, groups = _proj_builder(
                nc, tc, misc, dma_pool, big_ps, embT_d, wihcT, biasc, biascr,
                gi, t_len, KE, act, alu, f16, f32, "proj")
            emit_prefix()

            hist = hpool.tile([128, KH, BPC, t_len], f16, tag="hist")
            rz_ps = rz_psp.tile([128, 4, BPC], f32, tag="rz")
            n_ps = n_psp.tile([128, KH, BPC], f32, tag="n")

            # prime has_written for the preload+accumulate banks
            nc.tensor.matmul(rz_ps[:], zs[:], zx[:], start=True, stop=True)
            nc.tensor.matmul(n_ps[:], zs[:], zx[0:1, 0: KH * BPC], start=True, stop=True)
            # initial preloads for t=0
            nc.vector.tensor_copy(rz_ps[:], gi[:, 0:4, :, 0])
            nc.scalar.activation(n_ps[:], bhhnc_rep[:], act.Identity)

            def emit_step(t):
                if t > 0:
                    h_prev = hist[:, :, :, t - 1]
                    for c in (0, 1, 2, 3, 4, 5):
                        dst = rz_ps[:, c, :] if c < 4 else n_ps[:, c - 4, :]
                        for k in range(KH):
                            nc.tensor.matmul(
                                dst, whh16[:, k, c * 128: (c + 1) * 128],
                                h_prev[:, k, :],
                                start=False, stop=(k == KH - 1),
                                skip_group_check=True,
                            )
                rz = sb_pool.tile([128, 4, BPC], f32, tag="rz")
                tmp = sb_pool.tile([128, KH, BPC], f32, tag="tmp")
                nc.scalar.activation(rz[:], rz_ps[:], act.Sigmoid)
                nc.vector.tensor_tensor(tmp[:], n_ps[:], rz[:, 0:KH, :], alu.mult)
                nc.gpsimd.tensor_tensor(tmp[:], tmp[:], gi[:, 4:GC, :, t], alu.add)
                nn_ = sb_pool.tile([128, KH, BPC], f32, tag="nn")
                nc.scalar.activation(nn_[:], tmp[:], act.Tanh)
                d = sb_pool.tile([128, KH, BPC], f32, tag="dd")
                if t > 0:
                    nc.gpsimd.tensor_tensor(d[:], hist[:, :, :, t - 1], nn_[:], alu.subtract)
                else:
                    nc.gpsimd.tensor_scalar(d[:], nn_[:], -1.0, None, alu.mult)
                nc.vector.tensor_tensor(d[:], rz[:, 2:4, :], d[:], alu.mult)
                nc.gpsimd.tensor_tensor(hist[:, :, :, t], nn_[:], d[:], alu.add)
                # off-chain preloads for step t+1 (after this step's readers)
                if t + 1 < t_len:
                    nc.vector.tensor_copy(rz_ps[:], gi[:, 0:4, :, t + 1])
                    nc.scalar.activation(n_ps[:], bhhnc_rep[:], act.Identity)

            gidx = 0
            for t in range(t_len):
                emit_step(t)
                if t % 2 == 1 and gidx < len(groups):
                    groups[gidx]()
                    gidx += 1
            while gidx < len(groups):
                groups[gidx]()
                gidx += 1

            # ---- batched decisions: ks[b,t] = (h_t . wdiff > ncdiff) ----
            ks_sb = misc.tile([1, BPC * t_len], f32, tag="kssb")
            for b in range(BPC):
                dps = big_ps.tile([1, t_len], f32, tag="bps")
                for k in range(KH):
                    nc.tensor.matmul(
                        dps[:], wdiffT[:, k, :], hist[:, k, b, :],
                        start=(k == 0), stop=(k == KH - 1),
                    )
                nc.vector.tensor_tensor(
                    ks_sb[0:1, b * t_len: (b + 1) * t_len], dps[:],
                    ncdiff[0:1, b * t_len: (b + 1) * t_len], alu.is_gt,
                )
            nc.sync.dma_start(ks_d[:], ks_sb[:])

    return _split_excess_waits(nc)


def build_kernel2(t2, kf3, kf4, kf5):
    """GRU0/GRU1 + convs + pooling + final linear at dynamic length t2.

    L0: hoisted chunked proj0 (gi0 in SBUF) + PSUM preloads (gi0_rz, bhhn0).
    L1: input projection FUSED into the per-step matmul burst (Wih1 @ o1[t]
    accumulates into the same PSUM group as Whh1 @ o2[t-1]); rz bias,
    bhh1_n and bih1_n preloaded into PSUM; lag is only 2 waves, no gi1
    buffer. Conv max-pool windows (kf*) are compile-time constants."""
    import concourse.tile as tile
    from concourse import mybir

    _apply_tile_patch()
    nc = _mk_nc()
    f32 = mybir.dt.float32
    f16 = mybir.dt.float16
    act = mybir.ActivationFunctionType
    alu = mybir.AluOpType
    LAG = 2
    HALF = t2 // 2

    nembT_d = nc.dram_tensor("nembT", [KE, 128, BPC * t2], f16, kind="ExternalInput").ap()
    wih0T_d = nc.dram_tensor("wih0T", [128, KE, 3 * H], f16, kind="ExternalInput").ap()
    whh0T_d = nc.dram_tensor("whh0T", [128, KH, 3 * H], f16, kind="ExternalInput").ap()
    bias0_d = nc.dram_tensor("bias0", [128, GC], f32, kind="ExternalInput").ap()
    bias0r_d = nc.dram_tensor("bias0r", [128, GC, HALF], f32, kind="ExternalInput").ap()
    bhhn0_d = nc.dram_tensor("bhhn0", [128, KH, BPC], f32, kind="ExternalInput").ap()
    wih1T_d = nc.dram_tensor("wih1T", [128, KH, 3 * H], f16, kind="ExternalInput").ap()
    whh1T_d = nc.dram_tensor("whh1T", [128, KH, 3 * H], f16, kind="ExternalInput").ap()
    b1rz_d = nc.dram_tensor("b1rz", [128, 4, BPC], f32, kind="ExternalInput").ap()
    bhh1n_d = nc.dram_tensor("bhh1n", [128, KH, BPC], f32, kind="ExternalInput").ap()
    bih1n_d = nc.dram_tensor("bih1n", [128, KH, BPC], f32, kind="ExternalInput").ap()
    vt_d = nc.dram_tensor("vt", [1, BPC * t2], f16, kind="ExternalInput").ap()
    cw_d = nc.dram_tensor("cw", [128, 12, KH, NF], f16, kind="ExternalInput").ap()
    cb_d = nc.dram_tensor("cb", [NF, 3], f32, kind="ExternalInput").ap()
    woutT_d = nc.dram_tensor("woutT", [NF, 3], f32, kind="ExternalInput").ap()
    bout_d = nc.dram_tensor("bout", [1, 1], f32, kind="ExternalInput").ap()
    out_d = nc.dram_tensor("out", [1, BPC], f32, kind="ExternalOutput").ap()

    FS = (3, 4, 5)
    KFS = (kf3, kf4, kf5)

    with tile.TileContext(nc) as tc:
        from contextlib import ExitStack

        with ExitStack() as ctx:
            wpool = ctx.enter_context(tc.tile_pool(name="weights", bufs=1))
            gipool = ctx.enter_context(tc.tile_pool(name="gi", bufs=1))
            opool = ctx.enter_context(tc.tile_pool(name="obuf", bufs=1))
            dma_pool = ctx.enter_context(tc.tile_pool(name="dma", bufs=2))
            big_ps = ctx.enter_context(tc.tile_pool(name="bigps", bufs=2, space="PSUM"))
            rz0_psp = ctx.enter_context(tc.tile_pool(name="rz0ps", bufs=1, space="PSUM"))
            n0_psp = ctx.enter_context(tc.tile_pool(name="n0ps", bufs=1, space="PSUM"))
            rz1_psp = ctx.enter_context(tc.tile_pool(name="rz1ps", bufs=1, space="PSUM"))
            n1g_psp = ctx.enter_context(tc.tile_pool(name="n1gps", bufs=1, space="PSUM"))
            n1i_psp = ctx.enter_context(tc.tile_pool(name="n1ips", bufs=1, space="PSUM"))
            sb_pool = ctx.enter_context(tc.tile_pool(name="gates", bufs=2))
            misc = ctx.enter_context(tc.tile_pool(name="misc", bufs=1))

            def _load(pool, dram, shape, tag, dt=f32):
                t_ = pool.tile(shape, dt, tag=tag)
                nc.sync.dma_start(t_[:], dram[:])
                return t_

            wih0T = _load(wpool, wih0T_d, [128, KE, 3 * H], "wih0", f16)
            whh0T = _load(wpool, whh0T_d, [128, KH, 3 * H], "whh0", f16)
            bias0 = _load(wpool, bias0_d, [128, GC], "bias0")
            bias0r = _load(wpool, bias0r_d, [128, GC, HALF], "bias0r")
            bhhn0 = _load(wpool, bhhn0_d, [128, KH, BPC], "bhhn0")
            wih1T = _load(wpool, wih1T_d, [128, KH, 3 * H], "wih1", f16)
            whh1T = _load(wpool, whh1T_d, [128, KH, 3 * H], "whh1", f16)
            b1rz = _load(wpool, b1rz_d, [128, 4, BPC], "b1rz")
            bhh1n = _load(wpool, bhh1n_d, [128, KH, BPC], "bhh1n")
            bih1n = _load(wpool, bih1n_d, [128, KH, BPC], "bih1n")
            cw = _load(wpool, cw_d, [128, 12, KH, NF], "cw", f16)
            cb = _load(misc, cb_d, [NF, 3], "cb")
            woutT = _load(misc, woutT_d, [NF, 3], "woutT")
            bout = _load(misc, bout_d, [1, 1], "bout")
            vt = _load(misc, vt_d, [1, BPC * t2], "vt", f16)
            zs = misc.tile([1, 128], f16, tag="zs")
            zx = misc.tile([1, 4 * BPC], f16, tag="zx")
            nc.vector.memset(zs[:], 0.0)
            nc.vector.memset(zx[:], 0.0)

            gi0 = gipool.tile([128, GC, BPC, t2], f16, tag="gi0")
            o1 = opool.tile([128, KH, BPC, t2], f16, tag="o1")
            o2 = opool.tile([128, KH, BPC, t2], f16, tag="o2")

            emit_prefix, groups = _proj_builder(
                nc, tc, misc, dma_pool, big_ps, nembT_d, wih0T, bias0, bias0r,
                gi0, t2, KE, act, alu, f16, f32, "proj0")
            emit_prefix()

            rz0 = rz0_psp.tile([128, 4, BPC], f32, tag="rz0")
            n0 = n0_psp.tile([128, KH, BPC], f32, tag="n0")
            rz1 = rz1_psp.tile([128, 4, BPC], f32, tag="rz1")
            n1g = n1g_psp.tile([128, KH, BPC], f32, tag="n1g")
            n1i = n1i_psp.tile([128, KH, BPC], f32, tag="n1i")

            for ps_t in (rz0, rz1):
                nc.tensor.matmul(ps_t[:], zs[:], zx[:], start=True, stop=True)
            for ps_t in (n0, n1g, n1i):
                nc.tensor.matmul(ps_t[:], zs[:], zx[0:1, 0: KH * BPC], start=True, stop=True)
            # initial preloads
            nc.vector.tensor_copy(rz0[:], gi0[:, 0:4, :, 0])
            nc.scalar.activation(n0[:], bhhn0[:], act.Identity)
            nc.vector.tensor_copy(rz1[:], b1rz[:])
            nc.scalar.activation(n1g[:], bhh1n[:], act.Identity)
            nc.scalar.activation(n1i[:], bih1n[:], act.Identity)

            def emit_l0(t):
                if t > 0:
                    h_prev = o1[:, :, :, t - 1]
                    for c in (0, 1, 2, 3, 4, 5):
                        dst = rz0[:, c, :] if c < 4 else n0[:, c - 4, :]
                        for k in range(KH):
                            nc.tensor.matmul(
                                dst, whh0T[:, k, c * 128: (c + 1) * 128],
                                h_prev[:, k, :],
                                start=False, stop=(k == KH - 1),
                                skip_group_check=True,
                            )
                rz = sb_pool.tile([128, 4, BPC], f32, tag="rz0s")
                tmp = sb_pool.tile([128, KH, BPC], f32, tag="tmp0s")
                nc.scalar.activation(rz[:], rz0[:], act.Sigmoid)
                nc.vector.tensor_tensor(tmp[:], n0[:], rz[:, 0:KH, :], alu.mult)
                nc.gpsimd.tensor_tensor(tmp[:], tmp[:], gi0[:, 4:GC, :, t], alu.add)
                nn_ = sb_pool.tile([128, KH, BPC], f32, tag="nn0s")
                nc.scalar.activation(nn_[:], tmp[:], act.Tanh)
                d = sb_pool.tile([128, KH, BPC], f32, tag="dd0s")
                if t > 0:
                    nc.gpsimd.tensor_tensor(d[:], o1[:, :, :, t - 1], nn_[:], alu.subtract)
                else:
                    nc.gpsimd.tensor_scalar(d[:], nn_[:], -1.0, None, alu.mult)
                nc.vector.tensor_tensor(d[:], rz[:, 2:4, :], d[:], alu.mult)
                nc.gpsimd.tensor_tensor(o1[:, :, :, t], nn_[:], d[:], alu.add)
                if t + 1 < t2:
                    nc.vector.tensor_copy(rz0[:], gi0[:, 0:4, :, t + 1])
                    nc.scalar.activation(n0[:], bhhn0[:], act.Identity)

            def emit_l1(t):
                # fused burst: Whh1 @ o2[t-1] (if t>0) + Wih1 @ o1[t]
                for c in (0, 1, 2, 3, 4, 5):
                    if c < 4:
                        dst, dsti = rz1[:, c, :], rz1[:, c, :]
                    else:
                        dst, dsti = n1g[:, c - 4, :], n1i[:, c - 4, :]
                    for k in range(KH):
                        if t > 0:
                            nc.tensor.matmul(
                                dst, whh1T[:, k, c * 128: (c + 1) * 128],
                                o2[:, k, :, t - 1],
                                start=False, stop=False, skip_group_check=True,
                            )
                        nc.tensor.matmul(
                            dsti, wih1T[:, k, c * 128: (c + 1) * 128],
                            o1[:, k, :, t],
                            start=False, stop=(k == KH - 1), skip_group_check=True,
                        )
                rz = sb_pool.tile([128, 4, BPC], f32, tag="rz1s")
                tmp = sb_pool.tile([128, KH, BPC], f32, tag="tmp1s")
                nc.scalar.activation(rz[:], rz1[:], act.Sigmoid)
                nc.vector.tensor_tensor(tmp[:], n1g[:], rz[:, 0:KH, :], alu.mult)
                nc.vector.tensor_tensor(tmp[:], tmp[:], n1i[:], alu.add)
                nn_ = sb_pool.tile([128, KH, BPC], f32, tag="nn1s")
                nc.scalar.activation(nn_[:], tmp[:], act.Tanh)
                d = sb_pool.tile([128, KH, BPC], f32, tag="dd1s")
                if t > 0:
                    nc.gpsimd.tensor_tensor(d[:], o2[:, :, :, t - 1], nn_[:], alu.subtract)
                else:
                    nc.gpsimd.tensor_scalar(d[:], nn_[:], -1.0, None, alu.mult)
                nc.vector.tensor_tensor(d[:], rz[:, 2:4, :], d[:], alu.mult)
                nc.gpsimd.tensor_tensor(o2[:, :, :, t], nn_[:], d[:], alu.add)
                if t + 1 < t2:
                    nc.vector.tensor_copy(rz1[:], b1rz[:])
                    nc.scalar.activation(n1g[:], bhh1n[:], act.Identity)
                    nc.scalar.activation(n1i[:], bih1n[:], act.Identity)

            gidx = 0
            for w in range(t2 + LAG):
                if w < t2:
                    emit_l0(w)
                    if w % 2 == 1 and gidx < len(groups):
                        groups[gidx]()
                        gidx += 1
                if w >= LAG:
                    emit_l1(w - LAG)
            while gidx < len(groups):
                groups[gidx]()
                gidx += 1

            # ---- zero o2 past new_lens: o2 *= vt ----
            # partition-broadcast vt via a K=1 ones-matmul (PE outer product)
            ones_sb = misc.tile([1, 128], f16, tag="ones")
            nc.vector.memset(ones_sb[:], 1.0)
            for b in range(BPC):
                vtb = big_ps.tile([128, t2], f32, tag="bps")
                nc.tensor.matmul(
                    vtb[:], ones_sb[:], vt[0:1, b * t2: (b + 1) * t2],
                    start=True, stop=True,
                )
                for k in range(KH):
                    nc.vector.tensor_tensor(
                        o2[:, k, b, :], o2[:, k, b, :], vtb[:], alu.mult
                    )

            # ---- convs + relu + max-pool over compile-time window ----
            pooled = misc.tile([NF, 3, BPC], f32, tag="pooled")
            for b in range(BPC):
                for fi, fs in enumerate(FS):
                    nw = t2 - fs + 1
                    kf = KFS[fi]
                    ps = big_ps.tile([NF, t2], f32, tag="bps")
                    m0 = sum(FS[:fi])  # flat (fs,dt) base index
                    first = True
                    for dt_ in range(fs):
                        for k in range(KH):
                            nc.tensor.matmul(
                                ps[:, :nw],
                                cw[:, m0 + dt_, k, :],
                                o2[:, k, b, dt_: dt_ + nw],
                                start=first,
                                stop=(dt_ == fs - 1 and k == KH - 1),
                            )
                            first = False
                    crelu = sb_pool.tile([NF, t2], f32, tag="crelu")
                    nc.scalar.activation(
                        crelu[:, :kf], ps[:, :kf], act.Relu, bias=cb[:, fi: fi + 1]
                    )
                    nc.vector.tensor_reduce(
                        pooled[:, fi, b: b + 1], crelu[:, :kf],
                        mybir.AxisListType.X, alu.max,
                    )

            # ---- final linear ----
            fps = big_ps.tile([1, BPC], f32, tag="bps")
            for fi in range(3):
                nc.tensor.matmul(
                    fps[:], woutT[:, fi: fi + 1], pooled[:, fi, :],
                    start=(fi == 0), stop=(fi == 2),
                )
            out_sb = misc.tile([1, BPC], f32, tag="outsb")
            nc.scalar.activation(out_sb[:], fps[:], act.Identity, bias=bout[0:1, 0:1])
            nc.sync.dma_start(out_d[:], out_sb[:])

    return _split_excess_waits(nc)


# ------------------------------------------------------------- host orchestration
def _host_pack_k1(inputs, gumbel, t_len=T):
    f16 = _np_f16()
    emb = np.asarray(inputs["embedded"], np.float32)
    mask = np.asarray(inputs["mask"])
    lens = mask.sum(1)
    maxlen = int(lens.max())

    wihcT, whhT, biasc, bhhnc = _pack_gru_weights(
        inputs["Wih_c"], inputs["Whh_c"], inputs["bih_c"], inputs["bhh_c"])
    wdiff = (inputs["Wsel"][1] - inputs["Wsel"][0]).astype(np.float32)
    wdiffT = np.ascontiguousarray(wdiff.reshape(KH, 128).T[:, :, None])
    bdiff = float(inputs["bsel"][1] - inputs["bsel"][0])

    # ncdiff[b, t]: k_t = (h.wdiff > ncdiff); forced off when t >= maxlen-1
    ncdiff = np.full((B, t_len), 1.0e30, np.float32)
    upto = min(maxlen - 1, t_len)
    for t in range(1, upto):
        ncdiff[:, t] = -(bdiff + gumbel[t - 1, :, 1] - gumbel[t - 1, :, 0])

    biascr = np.ascontiguousarray(
        np.broadcast_to(biasc[:, :, None], (128, GC, t_len // 2))).astype(np.float32)

    in_maps = []
    for c in range(NCORES):
        rows = slice(c * BPC, (c + 1) * BPC)
        in_maps.append({
            "embT": _pack_embT(emb[rows, :t_len], t_len).astype(f16),
            "wihcT": wihcT.astype(f16),
            "biasc": biasc,
            "biascr": biascr,
            "whh16": whhT.astype(f16),
            "bhhnc": bhhnc,
            "wdiffT": wdiffT.astype(f16),
            "ncdiff": np.ascontiguousarray(
                ncdiff[rows].reshape(1, BPC * t_len)),
        })
    return in_maps, lens, maxlen


def _host_compact(inputs, ks_full, lens, maxlen, t_len=T):
    """ks_full: [B, t_len] decision bits (row t=0 ignored; selected[:,0]=1)."""
    emb = np.asarray(inputs["embedded"], np.float32)
    selected = np.zeros((B, t_len), np.int64)
    selected[:, 0] = 1
    selected[:, 1:] = ks_full[:, 1:]
    pos = np.arange(t_len)
    sel_valid = np.where(pos[None, :] < (lens - 1)[:, None], selected, 0)
    new_mask = np.where(pos[None, :] == (lens - 1)[:, None], 1, sel_valid)
    new_lens = new_mask.sum(1)
    Ldyn = max(int(new_lens.max()), 7)

    t2 = max(-(-Ldyn // 64) * 64, 64)
    new_emb = np.zeros((B, t2, E), np.float32)
    for b in range(B):
        idx = np.nonzero(new_mask[b])[0]
        new_emb[b, : len(idx)] = emb[b, idx]
    return new_emb, new_lens, Ldyn, t2


def _host_pack_k2(inputs, new_emb, new_lens, Ldyn, t2):
    f16 = _np_f16()
    wih0T, whh0T, bias0, bhhn0 = _pack_gru_weights(
        inputs["Wih0"], inputs["Whh0"], inputs["bih0"], inputs["bhh0"])
    wih1T, whh1T, bias1, bhhn1 = _pack_gru_weights(
        inputs["Wih1"], inputs["Whh1"], inputs["bih1"], inputs["bhh1"])

    FS = (3, 4, 5)
    cw = np.zeros((128, 12, KH, NF), np.float32)
    cb = np.zeros((NF, 3), np.float32)
    m = 0
    for fi, fs in enumerate(FS):
        w = np.asarray(inputs[f"conv_w{fs}"], np.float32)  # [NF,1,fs,H]
        cb[:, fi] = np.asarray(inputs[f"conv_b{fs}"], np.float32)
        for dt_ in range(fs):
            wt = w[:, 0, dt_, :].T  # [H, NF]
            cw[:, m, :, :] = wt.reshape(KH, 128, NF).transpose(1, 0, 2)
            m += 1

    woutT = np.ascontiguousarray(
        np.asarray(inputs["Wout"], np.float32)[0].reshape(3, NF).T)
    bout = np.asarray(inputs["bout"], np.float32).reshape(1, 1)

    vt_full = (np.arange(t2)[None, :] < new_lens[:, None]).astype(np.float32)

    bias0r = np.ascontiguousarray(
        np.broadcast_to(bias0[:, :, None], (128, GC, t2 // 2))).astype(np.float32)
    b1rz = np.ascontiguousarray(
        np.broadcast_to(bias1[:, 0:4, None], (128, 4, BPC))).astype(np.float32)
    bhh1 = np.asarray(inputs["bhh1"], np.float32)
    bih1 = np.asarray(inputs["bih1"], np.float32)
    bih1n = np.ascontiguousarray(np.broadcast_to(
        _pack_bias(bih1[2 * H:])[:, :, None], (128, KH, BPC))).astype(np.float32)

    in_maps = []
    for c in range(NCORES):
        rows = slice(c * BPC, (c + 1) * BPC)
        in_maps.append({
            "nembT": _pack_embT(new_emb[rows], t2).astype(f16),
            "wih0T": wih0T.astype(f16), "whh0T": whh0T.astype(f16),
            "bias0": bias0, "bias0r": bias0r, "bhhn0": bhhn0,
            "wih1T": wih1T.astype(f16), "whh1T": whh1T.astype(f16),
            "b1rz": b1rz, "bhh1n": bhhn1, "bih1n": bih1n,
            "vt": np.ascontiguousarray(
                vt_full[rows].reshape(1, BPC * t2)).astype(f16),
            "cw": cw.astype(f16), "cb": cb,
            "woutT": woutT, "bout": bout,
        })
    return in_maps


_NC_CACHE = {}


def _get_nc1(t_len=T):
    key = (1, t_len)
    if key not in _NC_CACHE:
        _NC_CACHE[key] = build_kernel1(t_len)
    return _NC_CACHE[key]


def _get_nc2(t2, kfs):
    key = (2, t2, kfs)
    if key not in _NC_CACHE:
        _NC_CACHE[key] = build_kernel2(t2, *kfs)
    return _NC_CACHE[key]


TRACE = False  # set True (with an NTFF hook registered) to collect exec times
LAST_STATS = {}


def kernel(**inputs):
    from concourse import bass_utils

    gumbel = _gumbel_cpu()
    core_ids = list(range(NCORES))

    in_maps1, lens, maxlen = _host_pack_k1(inputs, gumbel)
    nc1 = _get_nc1()
    res1 = bass_utils.run_bass_kernel_spmd(nc1, in_maps1, core_ids, trace=TRACE)
    ks_full = np.concatenate(
        [res1.results[c]["ks"].reshape(BPC, T) for c in range(NCORES)], axis=0)

    new_emb, new_lens, Ldyn, t2 = _host_compact(inputs, ks_full, lens, maxlen)
    kfs = tuple(min(Ldyn - fs + 1, t2 - fs + 1) for fs in (3, 4, 5))
    in_maps2 = _host_pack_k2(inputs, new_emb, new_lens, Ldyn, t2)
    nc2 = _get_nc2(t2, kfs)
    res2 = bass_utils.run_bass_kernel_spmd(nc2, in_maps2, core_ids, trace=TRACE)
    out = np.concatenate([res2.results[c]["out"][0] for c in range(NCORES)], axis=0)
    LAST_STATS["k1_ns"] = res1.exec_time_ns
    LAST_STATS["k2_ns"] = res2.exec_time_ns
    LAST_STATS["ks"] = ks_full
    LAST_STATS["new_lens"] = new_lens
    return out.astype(np.float32)


# revision 7
# speedup vs baseline: 1.5228x; 1.0464x over previous
"""Trainium2 Bass kernel for nn_CNN_RNN_88347477278730.

Pipeline (data-parallel over batch, 8 rows per core on 8 cores):
  kernel1 (device): chunked fp16 input projection (half hoisted, half
      interleaved into the early recurrence), then the 512-step
      select-policy GRUCell recurrence in full fp16 state with two 4-row
      batch streams for ILP; decisions batched into matmuls + is_gt at
      the end.
  host: compaction (gather kept tokens to the front), new_lens, Ldyn.
  kernel2 (device): compiled per dynamic sequence-length bucket t2
      (multiple of 32 >= max(new_lens)); chunked proj0, 2-layer GRU
      recurrence pipelined with a small lag, per-chunk proj1, Kim-CNN
      convs as shifted matmuls with compile-time pool windows, final
      linear.

All recurrence matmuls are gate-major (lhsT = weight tiles [K=128,
M=128], moving operand = h [K, batch]) so gate tensors land
partition-major where the elementwise engines are fast. The per-step
elementwise chain is 9 ops balanced across Vector/Scalar/GpSimd.
"""

import os
import subprocess
import sys
import tempfile

import numpy as np

# ---------------------------------------------------------------- constants
B, T, E, H, NF = 64, 512, 768, 256, 100
NCORES = 8
BPC = B // NCORES  # batch rows per core
KE = E // 128      # 6 K-tiles over the embedding dim
KH = H // 128      # 2 K-tiles over the hidden dim
GC = (3 * H) // 128  # 6 gate chunks (r: 0-1, z: 2-3, n: 4-5)

_F32 = None  # set lazily to mybir.dt.float32


# ------------------------------------------------------------- tile patch
def _apply_tile_patch():
    """This walrus build rejects >2 sem waits on one SP control instruction;
    split the TileContext tail drain into several drains of <=2 waits."""
    import concourse.tile as tile
    from concourse.vector_clock import ScopedClock, VectorClock

    if getattr(tile.TileContext, "_drain_split_patched", False):
        return

    def _patched(self, tick_clock, wait_clock):
        gc = tick_clock.global_clock
        n = len(gc)
        for start in range(0, n, 1):
            vec = [0] * n
            any_set = False
            for p in range(start, min(start + 1, n)):
                vec[p] = gc[p]
                any_set = any_set or vec[p] > 0
            if not any_set:
                continue
            d = self.nc.sync.drain()
            wait_clock.add_sem_waits(d.ins, ScopedClock({None: VectorClock(vec)}))
        self.nc.all_engine_barrier()
        assert self.sems is not None
        popped = self.nc._tile_sem_poison_stack.pop()
        assert popped is self._sem_poison
        self.nc.clear_and_free_semaphores(list(self.sems.allocated().values()))
        self.nc.all_engine_barrier()

    tile.TileContext._drain_and_barrier = _patched
    tile.TileContext._drain_split_patched = True


# ------------------------------------------------------------- gumbel (CPU)
def _gumbel_cpu():
    """jax.random.gumbel(key(42), (T-1, B, 2), f32) — computed in a CPU-jax
    subprocess so the accelerator backend is never involved (it must be
    bit-identical to the reference's CPU computation)."""
    path = os.path.join(tempfile.mkdtemp(), "gumbel.npy")
    code = (
        "import numpy as np, jax, jax.numpy as jnp\n"
        f"g = jax.random.gumbel(jax.random.key(42), ({T - 1}, {B}, 2), jnp.float32)\n"
        f"np.save({path!r}, np.asarray(g))\n"
    )
    env = dict(os.environ)
    env["TRN_TERMINAL_POOL_IPS"] = ""
    env["JAX_PLATFORMS"] = "cpu"
    extra = [p for p in sys.path if p and os.path.isdir(p)]
    env["PYTHONPATH"] = os.pathsep.join(extra)
    subprocess.run([sys.executable, "-c", code], env=env, check=True, capture_output=True)
    return np.load(path)


# ------------------------------------------------------------- host packing
def _pack_T(a2d):
    """[rows(=128*k), cols] -> [128, k, cols] weight-tile layout."""
    rows, cols = a2d.shape
    k = rows // 128
    return np.ascontiguousarray(a2d.reshape(k, 128, cols).transpose(1, 0, 2)).astype(np.float32)


def _pack_bias(b1d):
    """[128*k] -> [128, k]"""
    k = b1d.shape[0] // 128
    return np.ascontiguousarray(b1d.reshape(k, 128).T).astype(np.float32)


def _pack_embT(emb_rows, t_len=T):
    """[bpc, T, E] -> [KE, 128, bpc*T] (e-major tiles, free dims (b, t))."""
    bpc = emb_rows.shape[0]
    x = emb_rows.transpose(2, 0, 1).reshape(KE, 128, bpc * t_len)
    return np.ascontiguousarray(x).astype(np.float32)


def _pack_gru_weights(Wih, Whh, bih, bhh):
    """Returns (wihT, whhT, bias_proj, bhhn_rep) packings.

    bias_proj folds bih+bhh for the r,z chunks (added once at projection
    time); n chunks get bih only, with bhh_n applied per-step (it must be
    added to h@Whh_n *before* the r* multiply)."""
    wihT = _pack_T(np.ascontiguousarray(Wih.T))  # [128, KE or KH, 3H]
    whhT = _pack_T(np.ascontiguousarray(Whh.T))  # [128, KH, 3H]
    bias = np.empty(3 * H, np.float32)
    bias[: 2 * H] = bih[: 2 * H] + bhh[: 2 * H]
    bias[2 * H:] = bih[2 * H:]
    bias_proj = _pack_bias(bias)  # [128, GC]
    bhhn = _pack_bias(bhh[2 * H:])  # [128, KH]
    bhhn_rep = np.ascontiguousarray(
        np.broadcast_to(bhhn[:, :, None], (128, KH, BPC))
    ).astype(np.float32)
    return wihT, whhT, bias_proj, bhhn_rep


def _np_f16():
    from concourse import mybir

    return mybir.dt.np(mybir.dt.float16)


# ------------------------------------------------------------- bass builders
def _mk_nc():
    import concourse.bass as bass

    return bass.Bass("TRN2", target_bir_lowering=False, debug=False, num_devices=1)


def _split_excess_waits(nc, max_waits=1):
    """This walrus build can only encode ~2 sem waits per instruction
    (setupSyncWait 'Too many sync wait commands'). Hoist excess waits onto
    same-engine NoOps inserted just before the over-subscribed instruction;
    engine queues execute in order, so the wait semantics are identical."""
    from concourse import mybir

    nid = [0]
    for f in nc.m.functions:
        for bb in f.blocks:
            out = []
            changed = False
            for inst in bb.instructions:
                si = inst.sync_info
                lim = max_waits
                if si is not None and si.on_wait and len(si.on_wait) > lim:
                    waits = list(si.on_wait)
                    extra, keep = waits[:-lim], waits[-lim:]
                    for j in range(0, len(extra), max_waits):
                        nop = mybir.InstNoOp(
                            name=f"I-waitnop-{nid[0]}", ins=[], outs=[])
                        nid[0] += 1
                        nop.engine = inst.engine
                        nop.sync_info = mybir.SyncInfo(
                            on_wait=extra[j: j + max_waits], on_update=[])
                        nc.register_instruction(nop, overwrite=True)
                        out.append(nop)
                    inst.sync_info = mybir.SyncInfo(
                        on_wait=keep, on_update=list(si.on_update or []))
                    changed = True
                out.append(inst)
            if changed:
                bb.instructions = out
    return nc


def _proj_builder(nc, tc, misc_pool, dma_pool, big_ps, src_dram, wihT, biasc,
                  biasc_rep, gi, t_len, n_k, act, alu, f16, f32, tag):
    """Returns (emit_prefix, groups): chunked input projection.

    emit_prefix() emits the first t-half; `groups` is a list of closures,
    each emitting one (b, c) group of the second t-half (to be interleaved
    into the early recurrence steps). Copies alternate ACT (per-partition
    bias) / DVE (bias_rep tensor) to balance engines."""
    HALF = t_len // 2
    src_tiles = {}

    def _src(hb, b):
        key = (hb, b)
        if key not in src_tiles:
            s = dma_pool.tile([128, n_k, HALF], f16, tag=f"{tag}src")
            for k in range(n_k):
                nc.sync.dma_start(
                    s[:, k, :],
                    src_dram[k, :, b * t_len + hb * HALF: b * t_len + hb * HALF + HALF],
                )
            src_tiles[key] = s
        return src_tiles[key]

    def _group(hb, b, c):
        src = _src(hb, b)
        ps = big_ps.tile([128, HALF], f32, tag="bps")
        for k in range(n_k):
            nc.tensor.matmul(
                ps[:], wihT[:, k, c * 128: (c + 1) * 128], src[:, k, :],
                start=(k == 0), stop=(k == n_k - 1),
            )
        t0 = hb * HALF
        dst = gi[:, c, b, t0: t0 + HALF]
        if (b * GC + c) % 2 == 0:
            nc.scalar.activation(dst, ps[:], act.Identity, bias=biasc[:, c: c + 1])
        else:
            nc.vector.tensor_tensor(dst, ps[:], biasc_rep[:, c, :], alu.add)

    def emit_prefix():
        for b in range(BPC):
            for c in range(GC):
                _group(0, b, c)

    groups = [
        (lambda b=b, c=c: _group(1, b, c))
        for b in range(BPC) for c in range(GC)
    ]
    return emit_prefix, groups


def build_kernel1(t_len=T):
    """Select-policy kernel: fp16 everywhere; ONE merged 8-row stream (the
    per-step serial chain latency is the period — extra streams only add
    engine-queue coupling); gi_rz and bhh_n preloaded into PSUM off-chain so
    the gate matmuls accumulate straight onto them (start=False after a
    has_written-priming dummy matmul); 7-op chain; decisions batched at
    the end."""
    import concourse.tile as tile
    from concourse import mybir

    _apply_tile_patch()
    nc = _mk_nc()
    f32 = mybir.dt.float32
    f16 = mybir.dt.float16
    act = mybir.ActivationFunctionType
    alu = mybir.AluOpType
    HALF = t_len // 2

    embT_d = nc.dram_tensor("embT", [KE, 128, BPC * t_len], f16, kind="ExternalInput").ap()
    wihcT_d = nc.dram_tensor("wihcT", [128, KE, 3 * H], f16, kind="ExternalInput").ap()
    biasc_d = nc.dram_tensor("biasc", [128, GC], f32, kind="ExternalInput").ap()
    biascr_d = nc.dram_tensor("biascr", [128, GC, HALF], f32, kind="ExternalInput").ap()
    whh16_d = nc.dram_tensor("whh16", [128, KH, 3 * H], f16, kind="ExternalInput").ap()
    bhhnc_d = nc.dram_tensor("bhhnc", [128, KH, BPC], f32, kind="ExternalInput").ap()
    wdiffT_d = nc.dram_tensor("wdiffT", [128, KH, 1], f16, kind="ExternalInput").ap()
    ncdiff_d = nc.dram_tensor("ncdiff", [1, BPC * t_len], f32, kind="ExternalInput").ap()
    ks_d = nc.dram_tensor("ks", [1, BPC * t_len], f32, kind="ExternalOutput").ap()

    with tile.TileContext(nc) as tc:
        from contextlib import ExitStack

        with ExitStack() as ctx:
            wpool = ctx.enter_context(tc.tile_pool(name="weights", bufs=1))
            gipool = ctx.enter_context(tc.tile_pool(name="gi", bufs=1))
            hpool = ctx.enter_context(tc.tile_pool(name="hist", bufs=1))
            dma_pool = ctx.enter_context(tc.tile_pool(name="dma", bufs=2))
            big_ps = ctx.enter_context(tc.tile_pool(name="bigps", bufs=2, space="PSUM"))
            rz_psp = ctx.enter_context(tc.tile_pool(name="rzps", bufs=1, space="PSUM"))
            n_psp = ctx.enter_context(tc.tile_pool(name="nps", bufs=1, space="PSUM"))
            sb_pool = ctx.enter_context(tc.tile_pool(name="gates", bufs=2))
            misc = ctx.enter_context(tc.tile_pool(name="misc", bufs=1))

            def _load(pool, dram, shape, tag, dt=f32):
                t_ = pool.tile(shape, dt, tag=tag)
                nc.sync.dma_start(t_[:], dram[:])
                return t_

            wihcT = _load(wpool, wihcT_d, [128, KE, 3 * H], "wihcT", f16)
            biasc = _load(wpool, biasc_d, [128, GC], "biasc")
            biascr = _load(wpool, biascr_d, [128, GC, HALF], "biascr")
            whh16 = _load(wpool, whh16_d, [128, KH, 3 * H], "whh16", f16)
            bhhnc_rep = _load(wpool, bhhnc_d, [128, KH, BPC], "bhhnc")
            wdiffT = _load(misc, wdiffT_d, [128, KH, 1], "wdiffT", f16)
            ncdiff = _load(misc, ncdiff_d, [1, BPC * t_len], "ncdiff")
            zs = misc.tile([1, 128], f16, tag="zs")
            zx = misc.tile([1, 4 * BPC], f16, tag="zx")
            nc.vector.memset(zs[:], 0.0)
            nc.vector.memset(zx[:], 0.0)

            # gi layout: [128, GC, BPC, t] (t innermost: contiguous proj copies)
            gi = gipool.tile([128, GC, BPC, t_len], f16, tag="gi")
            emit_prefix, groups = _proj_builder(
                nc, tc, misc, dma_pool, big_ps, embT_d, wihcT, biasc, biascr,
                gi, t_len, KE, act, alu, f16, f32, "proj")
            emit_prefix()

            hist = hpool.tile([128, KH, BPC, t_len], f16, tag="hist")
            rz_ps = rz_psp.tile([128, 4, BPC], f32, tag="rz")
            n_ps = n_psp.tile([128, KH, BPC], f32, tag="n")

            # prime has_written for the preload+accumulate banks
            nc.tensor.matmul(rz_ps[:], zs[:], zx[:], start=True, stop=True)
            nc.tensor.matmul(n_ps[:], zs[:], zx[0:1, 0: KH * BPC], start=True, stop=True)
            # initial preloads for t=0
            nc.vector.tensor_copy(rz_ps[:], gi[:, 0:4, :, 0])
            nc.scalar.activation(n_ps[:], bhhnc_rep[:], act.Identity)

            def _gh(w, rhs, last):
                """Accumulate Whh @ rhs into the gate banks (start=False)."""
                for c in (0, 1, 2, 3, 4, 5):
                    dst = rz_ps[:, c, :] if c < 4 else n_ps[:, c - 4, :]
                    for k in range(KH):
                        nc.tensor.matmul(
                            dst, w[:, k, c * 128: (c + 1) * 128], rhs[:, k, :],
                            start=False, stop=(last and k == KH - 1),
                            skip_group_check=True,
                        )

            def emit_step(t):
                # gh(t) was already accumulated by step t-1 (MM-split: the
                # Whh@nn part right after tanh, the Whh@d part after op8).
                rz = sb_pool.tile([128, 4, BPC], f32, tag="rz")
                tmp = sb_pool.tile([128, KH, BPC], f32, tag="tmp")
                nc.scalar.activation(rz[:], rz_ps[:], act.Sigmoid)
                if t + 1 < t_len:
                    nc.vector.tensor_copy(rz_ps[:], gi[:, 0:4, :, t + 1])
                nc.vector.tensor_tensor(tmp[:], n_ps[:], rz[:, 0:KH, :], alu.mult)
                if t + 1 < t_len:
                    nc.scalar.activation(n_ps[:], bhhnc_rep[:], act.Identity)
                nc.gpsimd.tensor_tensor(tmp[:], tmp[:], gi[:, 4:GC, :, t], alu.add)
                nn16 = sb_pool.tile([128, KH, BPC], f16, tag="nn")
                nc.scalar.activation(nn16[:], tmp[:], act.Tanh)
                if t + 1 < t_len:
                    _gh(whh16, nn16, last=False)
                d = sb_pool.tile([128, KH, BPC], f32, tag="dd")
                if t > 0:
                    nc.gpsimd.tensor_tensor(d[:], hist[:, :, :, t - 1], nn16[:], alu.subtract)
                else:
                    nc.gpsimd.tensor_scalar(d[:], nn16[:], -1.0, None, alu.mult)
                d16 = sb_pool.tile([128, KH, BPC], f16, tag="dd16")
                nc.vector.tensor_tensor(d16[:], rz[:, 2:4, :], d[:], alu.mult)
                if t + 1 < t_len:
                    _gh(whh16, d16, last=True)
                nc.gpsimd.tensor_tensor(hist[:, :, :, t], nn16[:], d16[:], alu.add)

            gidx = 0
            for t in range(t_len):
                emit_step(t)
                if t % 2 == 1 and gidx < len(groups):
                    groups[gidx]()
                    gidx += 1
            while gidx < len(groups):
                groups[gidx]()
                gidx += 1

            # ---- batched decisions: ks[b,t] = (h_t . wdiff > ncdiff) ----
            ks_sb = misc.tile([1, BPC * t_len], f32, tag="kssb")
            for b in range(BPC):
                dps = big_ps.tile([1, t_len], f32, tag="bps")
                for k in range(KH):
                    nc.tensor.matmul(
                        dps[:], wdiffT[:, k, :], hist[:, k, b, :],
                        start=(k == 0), stop=(k == KH - 1),
                    )
                nc.vector.tensor_tensor(
                    ks_sb[0:1, b * t_len: (b + 1) * t_len], dps[:],
                    ncdiff[0:1, b * t_len: (b + 1) * t_len], alu.is_gt,
                )
            nc.sync.dma_start(ks_d[:], ks_sb[:])

    return _split_excess_waits(nc)


def build_kernel2(t2, kf3, kf4, kf5):
    """GRU0/GRU1 + convs + pooling + final linear at dynamic length t2.

    L0: hoisted chunked proj0 (gi0 in SBUF) + PSUM preloads (gi0_rz, bhhn0).
    L1: input projection FUSED into the per-step matmul burst (Wih1 @ o1[t]
    accumulates into the same PSUM group as Whh1 @ o2[t-1]); rz bias,
    bhh1_n and bih1_n preloaded into PSUM; lag is only 2 waves, no gi1
    buffer. Conv max-pool windows (kf*) are compile-time constants."""
    import concourse.tile as tile
    from concourse import mybir

    _apply_tile_patch()
    nc = _mk_nc()
    f32 = mybir.dt.float32
    f16 = mybir.dt.float16
    act = mybir.ActivationFunctionType
    alu = mybir.AluOpType
    LAG = 2
    HALF = t2 // 2

    nembT_d = nc.dram_tensor("nembT", [KE, 128, BPC * t2], f16, kind="ExternalInput").ap()
    wih0T_d = nc.dram_tensor("wih0T", [128, KE, 3 * H], f16, kind="ExternalInput").ap()
    whh0T_d = nc.dram_tensor("whh0T", [128, KH, 3 * H], f16, kind="ExternalInput").ap()
    bias0_d = nc.dram_tensor("bias0", [128, GC], f32, kind="ExternalInput").ap()
    bias0r_d = nc.dram_tensor("bias0r", [128, GC, HALF], f32, kind="ExternalInput").ap()
    bhhn0_d = nc.dram_tensor("bhhn0", [128, KH, BPC], f32, kind="ExternalInput").ap()
    wih1T_d = nc.dram_tensor("wih1T", [128, KH, 3 * H], f16, kind="ExternalInput").ap()
    whh1T_d = nc.dram_tensor("whh1T", [128, KH, 3 * H], f16, kind="ExternalInput").ap()
    b1rz_d = nc.dram_tensor("b1rz", [128, 4, BPC], f32, kind="ExternalInput").ap()
    bhh1n_d = nc.dram_tensor("bhh1n", [128, KH, BPC], f32, kind="ExternalInput").ap()
    bih1n_d = nc.dram_tensor("bih1n", [128, KH, BPC], f32, kind="ExternalInput").ap()
    vt_d = nc.dram_tensor("vt", [1, BPC * t2], f16, kind="ExternalInput").ap()
    cw_d = nc.dram_tensor("cw", [128, 12, KH, NF], f16, kind="ExternalInput").ap()
    cb_d = nc.dram_tensor("cb", [NF, 3], f32, kind="ExternalInput").ap()
    woutT_d = nc.dram_tensor("woutT", [NF, 3], f32, kind="ExternalInput").ap()
    bout_d = nc.dram_tensor("bout", [1, 1], f32, kind="ExternalInput").ap()
    out_d = nc.dram_tensor("out", [1, BPC], f32, kind="ExternalOutput").ap()

    FS = (3, 4, 5)
    KFS = (kf3, kf4, kf5)

    with tile.TileContext(nc) as tc:
        from contextlib import ExitStack

        with ExitStack() as ctx:
            wpool = ctx.enter_context(tc.tile_pool(name="weights", bufs=1))
            gipool = ctx.enter_context(tc.tile_pool(name="gi", bufs=1))
            opool = ctx.enter_context(tc.tile_pool(name="obuf", bufs=1))
            dma_pool = ctx.enter_context(tc.tile_pool(name="dma", bufs=2))
            big_ps = ctx.enter_context(tc.tile_pool(name="bigps", bufs=2, space="PSUM"))
            rz0_psp = ctx.enter_context(tc.tile_pool(name="rz0ps", bufs=1, space="PSUM"))
            n0_psp = ctx.enter_context(tc.tile_pool(name="n0ps", bufs=1, space="PSUM"))
            rz1_psp = ctx.enter_context(tc.tile_pool(name="rz1ps", bufs=1, space="PSUM"))
            n1g_psp = ctx.enter_context(tc.tile_pool(name="n1gps", bufs=1, space="PSUM"))
            n1i_psp = ctx.enter_context(tc.tile_pool(name="n1ips", bufs=1, space="PSUM"))
            sb_pool = ctx.enter_context(tc.tile_pool(name="gates", bufs=2))
            misc = ctx.enter_context(tc.tile_pool(name="misc", bufs=1))

            def _load(pool, dram, shape, tag, dt=f32):
                t_ = pool.tile(shape, dt, tag=tag)
                nc.sync.dma_start(t_[:], dram[:])
                return t_

            wih0T = _load(wpool, wih0T_d, [128, KE, 3 * H], "wih0", f16)
            whh0T = _load(wpool, whh0T_d, [128, KH, 3 * H], "whh0", f16)
            bias0 = _load(wpool, bias0_d, [128, GC], "bias0")
            bias0r = _load(wpool, bias0r_d, [128, GC, HALF], "bias0r")
            bhhn0 = _load(wpool, bhhn0_d, [128, KH, BPC], "bhhn0")
            wih1T = _load(wpool, wih1T_d, [128, KH, 3 * H], "wih1", f16)
            whh1T = _load(wpool, whh1T_d, [128, KH, 3 * H], "whh1", f16)
            b1rz = _load(wpool, b1rz_d, [128, 4, BPC], "b1rz")
            bhh1n = _load(wpool, bhh1n_d, [128, KH, BPC], "bhh1n")
            bih1n = _load(wpool, bih1n_d, [128, KH, BPC], "bih1n")
            cw = _load(wpool, cw_d, [128, 12, KH, NF], "cw", f16)
            cb = _load(misc, cb_d, [NF, 3], "cb")
            woutT = _load(misc, woutT_d, [NF, 3], "woutT")
            bout = _load(misc, bout_d, [1, 1], "bout")
            vt = _load(misc, vt_d, [1, BPC * t2], "vt", f16)
            zs = misc.tile([1, 128], f16, tag="zs")
            zx = misc.tile([1, 4 * BPC], f16, tag="zx")
            nc.vector.memset(zs[:], 0.0)
            nc.vector.memset(zx[:], 0.0)

            gi0 = gipool.tile([128, GC, BPC, t2], f16, tag="gi0")
            o1 = opool.tile([128, KH, BPC, t2], f16, tag="o1")
            o2 = opool.tile([128, KH, BPC, t2], f16, tag="o2")

            emit_prefix, groups = _proj_builder(
                nc, tc, misc, dma_pool, big_ps, nembT_d, wih0T, bias0, bias0r,
                gi0, t2, KE, act, alu, f16, f32, "proj0")
            emit_prefix()

            rz0 = rz0_psp.tile([128, 4, BPC], f32, tag="rz0")
            n0 = n0_psp.tile([128, KH, BPC], f32, tag="n0")
            rz1 = rz1_psp.tile([128, 4, BPC], f32, tag="rz1")
            n1g = n1g_psp.tile([128, KH, BPC], f32, tag="n1g")
            n1i = n1i_psp.tile([128, KH, BPC], f32, tag="n1i")

            for ps_t in (rz0, rz1):
                nc.tensor.matmul(ps_t[:], zs[:], zx[:], start=True, stop=True)
            for ps_t in (n0, n1g, n1i):
                nc.tensor.matmul(ps_t[:], zs[:], zx[0:1, 0: KH * BPC], start=True, stop=True)
            # initial preloads
            nc.vector.tensor_copy(rz0[:], gi0[:, 0:4, :, 0])
            nc.scalar.activation(n0[:], bhhn0[:], act.Identity)
            nc.vector.tensor_copy(rz1[:], b1rz[:])
            nc.scalar.activation(n1g[:], bhh1n[:], act.Identity)
            nc.scalar.activation(n1i[:], bih1n[:], act.Identity)
            proj1_0 = [True]

            def _gh0(rhs, last):
                for c in (0, 1, 2, 3, 4, 5):
                    dst = rz0[:, c, :] if c < 4 else n0[:, c - 4, :]
                    for k in range(KH):
                        nc.tensor.matmul(
                            dst, whh0T[:, k, c * 128: (c + 1) * 128], rhs[:, k, :],
                            start=False, stop=(last and k == KH - 1),
                            skip_group_check=True,
                        )

            def emit_l0(t):
                rz = sb_pool.tile([128, 4, BPC], f32, tag="rz0s")
                tmp = sb_pool.tile([128, KH, BPC], f32, tag="tmp0s")
                nc.scalar.activation(rz[:], rz0[:], act.Sigmoid)
                if t + 1 < t2:
                    nc.vector.tensor_copy(rz0[:], gi0[:, 0:4, :, t + 1])
                nc.vector.tensor_tensor(tmp[:], n0[:], rz[:, 0:KH, :], alu.mult)
                if t + 1 < t2:
                    nc.scalar.activation(n0[:], bhhn0[:], act.Identity)
                nc.gpsimd.tensor_tensor(tmp[:], tmp[:], gi0[:, 4:GC, :, t], alu.add)
                nn16 = sb_pool.tile([128, KH, BPC], f16, tag="nn0s")
                nc.scalar.activation(nn16[:], tmp[:], act.Tanh)
                if t + 1 < t2:
                    _gh0(nn16, last=False)
                d = sb_pool.tile([128, KH, BPC], f32, tag="dd0s")
                if t > 0:
                    nc.gpsimd.tensor_tensor(d[:], o1[:, :, :, t - 1], nn16[:], alu.subtract)
                else:
                    nc.gpsimd.tensor_scalar(d[:], nn16[:], -1.0, None, alu.mult)
                d16 = sb_pool.tile([128, KH, BPC], f16, tag="dd016")
                nc.vector.tensor_tensor(d16[:], rz[:, 2:4, :], d[:], alu.mult)
                if t + 1 < t2:
                    _gh0(d16, last=True)
                nc.gpsimd.tensor_tensor(o1[:, :, :, t], nn16[:], d16[:], alu.add)

            def _gh1(rhs, last):
                """Whh1 @ rhs into rz1/n1g."""
                for c in (0, 1, 2, 3, 4, 5):
                    dst = rz1[:, c, :] if c < 4 else n1g[:, c - 4, :]
                    for k in range(KH):
                        nc.tensor.matmul(
                            dst, whh1T[:, k, c * 128: (c + 1) * 128], rhs[:, k, :],
                            start=False, stop=(last and k == KH - 1),
                            skip_group_check=True,
                        )

            def _proj1(t):
                """Wih1 @ o1[t] into rz1/n1i (fused input projection)."""
                for c in (0, 1, 2, 3, 4, 5):
                    dst = rz1[:, c, :] if c < 4 else n1i[:, c - 4, :]
                    for k in range(KH):
                        nc.tensor.matmul(
                            dst, wih1T[:, k, c * 128: (c + 1) * 128],
                            o1[:, k, :, t],
                            start=False, stop=(k == KH - 1), skip_group_check=True,
                        )

            def emit_l1(t):
                rz = sb_pool.tile([128, 4, BPC], f32, tag="rz1s")
                tmp = sb_pool.tile([128, KH, BPC], f32, tag="tmp1s")
                nc.scalar.activation(rz[:], rz1[:], act.Sigmoid)
                if t + 1 < t2:
                    nc.vector.tensor_copy(rz1[:], b1rz[:])
                nc.vector.tensor_tensor(tmp[:], n1g[:], rz[:, 0:KH, :], alu.mult)
                if t + 1 < t2:
                    nc.scalar.activation(n1g[:], bhh1n[:], act.Identity)
                nc.vector.tensor_tensor(tmp[:], tmp[:], n1i[:], alu.add)
                if t + 1 < t2:
                    nc.scalar.activation(n1i[:], bih1n[:], act.Identity)
                    _proj1(t + 1)
                nn16 = sb_pool.tile([128, KH, BPC], f16, tag="nn1s")
                nc.scalar.activation(nn16[:], tmp[:], act.Tanh)
                if t + 1 < t2:
                    _gh1(nn16, last=False)
                d = sb_pool.tile([128, KH, BPC], f32, tag="dd1s")
                if t > 0:
                    nc.gpsimd.tensor_tensor(d[:], o2[:, :, :, t - 1], nn16[:], alu.subtract)
                else:
                    nc.gpsimd.tensor_scalar(d[:], nn16[:], -1.0, None, alu.mult)
                d16 = sb_pool.tile([128, KH, BPC], f16, tag="dd116")
                nc.vector.tensor_tensor(d16[:], rz[:, 2:4, :], d[:], alu.mult)
                if t + 1 < t2:
                    _gh1(d16, last=True)
                nc.gpsimd.tensor_tensor(o2[:, :, :, t], nn16[:], d16[:], alu.add)

            gidx = 0
            for w in range(t2 + LAG):
                if w < t2:
                    emit_l0(w)
                    if w % 2 == 1 and gidx < len(groups):
                        groups[gidx]()
                        gidx += 1
                if w >= LAG:
                    if proj1_0[0]:
                        _proj1(0)
                        proj1_0[0] = False
                    emit_l1(w - LAG)
            while gidx < len(groups):
                groups[gidx]()
                gidx += 1

            # ---- zero o2 past new_lens: o2 *= vt ----
            # partition-broadcast vt via a K=1 ones-matmul (PE outer product)
            ones_sb = misc.tile([1, 128], f16, tag="ones")
            nc.vector.memset(ones_sb[:], 1.0)
            for b in range(BPC):
                vtb = big_ps.tile([128, t2], f32, tag="bps")
                nc.tensor.matmul(
                    vtb[:], ones_sb[:], vt[0:1, b * t2: (b + 1) * t2],
                    start=True, stop=True,
                )
                for k in range(KH):
                    nc.vector.tensor_tensor(
                        o2[:, k, b, :], o2[:, k, b, :], vtb[:], alu.mult
                    )

            # ---- convs + relu + max-pool over compile-time window ----
            pooled = misc.tile([NF, 3, BPC], f32, tag="pooled")
            for b in range(BPC):
                for fi, fs in enumerate(FS):
                    nw = t2 - fs + 1
                    kf = KFS[fi]
                    ps = big_ps.tile([NF, t2], f32, tag="bps")
                    m0 = sum(FS[:fi])  # flat (fs,dt) base index
                    first = True
                    for dt_ in range(fs):
                        for k in range(KH):
                            nc.tensor.matmul(
                                ps[:, :nw],
                                cw[:, m0 + dt_, k, :],
                                o2[:, k, b, dt_: dt_ + nw],
                                start=first,
                                stop=(dt_ == fs - 1 and k == KH - 1),
                            )
                            first = False
                    crelu = sb_pool.tile([NF, t2], f32, tag="crelu")
                    nc.scalar.activation(
                        crelu[:, :kf], ps[:, :kf], act.Relu, bias=cb[:, fi: fi + 1]
                    )
                    nc.vector.tensor_reduce(
                        pooled[:, fi, b: b + 1], crelu[:, :kf],
                        mybir.AxisListType.X, alu.max,
                    )

            # ---- final linear ----
            fps = big_ps.tile([1, BPC], f32, tag="bps")
            for fi in range(3):
                nc.tensor.matmul(
                    fps[:], woutT[:, fi: fi + 1], pooled[:, fi, :],
                    start=(fi == 0), stop=(fi == 2),
                )
            out_sb = misc.tile([1, BPC], f32, tag="outsb")
            nc.scalar.activation(out_sb[:], fps[:], act.Identity, bias=bout[0:1, 0:1])
            nc.sync.dma_start(out_d[:], out_sb[:])

    return _split_excess_waits(nc)


# ------------------------------------------------------------- host orchestration
def _host_pack_k1(inputs, gumbel, t_len=T):
    f16 = _np_f16()
    emb = np.asarray(inputs["embedded"], np.float32)
    mask = np.asarray(inputs["mask"])
    lens = mask.sum(1)
    maxlen = int(lens.max())

    wihcT, whhT, biasc, bhhnc = _pack_gru_weights(
        inputs["Wih_c"], inputs["Whh_c"], inputs["bih_c"], inputs["bhh_c"])
    wdiff = (inputs["Wsel"][1] - inputs["Wsel"][0]).astype(np.float32)
    wdiffT = np.ascontiguousarray(wdiff.reshape(KH, 128).T[:, :, None])
    bdiff = float(inputs["bsel"][1] - inputs["bsel"][0])

    # ncdiff[b, t]: k_t = (h.wdiff > ncdiff); forced off when t >= maxlen-1
    ncdiff = np.full((B, t_len), 1.0e30, np.float32)
    upto = min(maxlen - 1, t_len)
    for t in range(1, upto):
        ncdiff[:, t] = -(bdiff + gumbel[t - 1, :, 1] - gumbel[t - 1, :, 0])

    biascr = np.ascontiguousarray(
        np.broadcast_to(biasc[:, :, None], (128, GC, t_len // 2))).astype(np.float32)

    in_maps = []
    for c in range(NCORES):
        rows = slice(c * BPC, (c + 1) * BPC)
        in_maps.append({
            "embT": _pack_embT(emb[rows, :t_len], t_len).astype(f16),
            "wihcT": wihcT.astype(f16),
            "biasc": biasc,
            "biascr": biascr,
            "whh16": whhT.astype(f16),
            "bhhnc": bhhnc,
            "wdiffT": wdiffT.astype(f16),
            "ncdiff": np.ascontiguousarray(
                ncdiff[rows].reshape(1, BPC * t_len)),
        })
    return in_maps, lens, maxlen


def _host_compact(inputs, ks_full, lens, maxlen, t_len=T):
    """ks_full: [B, t_len] decision bits (row t=0 ignored; selected[:,0]=1)."""
    emb = np.asarray(inputs["embedded"], np.float32)
    selected = np.zeros((B, t_len), np.int64)
    selected[:, 0] = 1
    selected[:, 1:] = ks_full[:, 1:]
    pos = np.arange(t_len)
    sel_valid = np.where(pos[None, :] < (lens - 1)[:, None], selected, 0)
    new_mask = np.where(pos[None, :] == (lens - 1)[:, None], 1, sel_valid)
    new_lens = new_mask.sum(1)
    Ldyn = max(int(new_lens.max()), 7)

    t2 = max(-(-Ldyn // 64) * 64, 64)
    new_emb = np.zeros((B, t2, E), np.float32)
    for b in range(B):
        idx = np.nonzero(new_mask[b])[0]
        new_emb[b, : len(idx)] = emb[b, idx]
    return new_emb, new_lens, Ldyn, t2


def _host_pack_k2(inputs, new_emb, new_lens, Ldyn, t2):
    f16 = _np_f16()
    wih0T, whh0T, bias0, bhhn0 = _pack_gru_weights(
        inputs["Wih0"], inputs["Whh0"], inputs["bih0"], inputs["bhh0"])
    wih1T, whh1T, bias1, bhhn1 = _pack_gru_weights(
        inputs["Wih1"], inputs["Whh1"], inputs["bih1"], inputs["bhh1"])

    FS = (3, 4, 5)
    cw = np.zeros((128, 12, KH, NF), np.float32)
    cb = np.zeros((NF, 3), np.float32)
    m = 0
    for fi, fs in enumerate(FS):
        w = np.asarray(inputs[f"conv_w{fs}"], np.float32)  # [NF,1,fs,H]
        cb[:, fi] = np.asarray(inputs[f"conv_b{fs}"], np.float32)
        for dt_ in range(fs):
            wt = w[:, 0, dt_, :].T  # [H, NF]
            cw[:, m, :, :] = wt.reshape(KH, 128, NF).transpose(1, 0, 2)
            m += 1

    woutT = np.ascontiguousarray(
        np.asarray(inputs["Wout"], np.float32)[0].reshape(3, NF).T)
    bout = np.asarray(inputs["bout"], np.float32).reshape(1, 1)

    vt_full = (np.arange(t2)[None, :] < new_lens[:, None]).astype(np.float32)

    bias0r = np.ascontiguousarray(
        np.broadcast_to(bias0[:, :, None], (128, GC, t2 // 2))).astype(np.float32)
    b1rz = np.ascontiguousarray(
        np.broadcast_to(bias1[:, 0:4, None], (128, 4, BPC))).astype(np.float32)
    bhh1 = np.asarray(inputs["bhh1"], np.float32)
    bih1 = np.asarray(inputs["bih1"], np.float32)
    bih1n = np.ascontiguousarray(np.broadcast_to(
        _pack_bias(bih1[2 * H:])[:, :, None], (128, KH, BPC))).astype(np.float32)

    in_maps = []
    for c in range(NCORES):
        rows = slice(c * BPC, (c + 1) * BPC)
        in_maps.append({
            "nembT": _pack_embT(new_emb[rows], t2).astype(f16),
            "wih0T": wih0T.astype(f16), "whh0T": whh0T.astype(f16),
            "bias0": bias0, "bias0r": bias0r, "bhhn0": bhhn0,
            "wih1T": wih1T.astype(f16), "whh1T": whh1T.astype(f16),
            "b1rz": b1rz, "bhh1n": bhhn1, "bih1n": bih1n,
            "vt": np.ascontiguousarray(
                vt_full[rows].reshape(1, BPC * t2)).astype(f16),
            "cw": cw.astype(f16), "cb": cb,
            "woutT": woutT, "bout": bout,
        })
    return in_maps


_NC_CACHE = {}


def _get_nc1(t_len=T):
    key = (1, t_len)
    if key not in _NC_CACHE:
        _NC_CACHE[key] = build_kernel1(t_len)
    return _NC_CACHE[key]


def _get_nc2(t2, kfs):
    key = (2, t2, kfs)
    if key not in _NC_CACHE:
        _NC_CACHE[key] = build_kernel2(t2, *kfs)
    return _NC_CACHE[key]


TRACE = False  # set True (with an NTFF hook registered) to collect exec times
LAST_STATS = {}


def kernel(**inputs):
    from concourse import bass_utils

    gumbel = _gumbel_cpu()
    core_ids = list(range(NCORES))

    in_maps1, lens, maxlen = _host_pack_k1(inputs, gumbel)
    nc1 = _get_nc1()
    res1 = bass_utils.run_bass_kernel_spmd(nc1, in_maps1, core_ids, trace=TRACE)
    ks_full = np.concatenate(
        [res1.results[c]["ks"].reshape(BPC, T) for c in range(NCORES)], axis=0)

    new_emb, new_lens, Ldyn, t2 = _host_compact(inputs, ks_full, lens, maxlen)
    kfs = tuple(min(Ldyn - fs + 1, t2 - fs + 1) for fs in (3, 4, 5))
    in_maps2 = _host_pack_k2(inputs, new_emb, new_lens, Ldyn, t2)
    nc2 = _get_nc2(t2, kfs)
    res2 = bass_utils.run_bass_kernel_spmd(nc2, in_maps2, core_ids, trace=TRACE)
    out = np.concatenate([res2.results[c]["out"][0] for c in range(NCORES)], axis=0)
    LAST_STATS["k1_ns"] = res1.exec_time_ns
    LAST_STATS["k2_ns"] = res2.exec_time_ns
    LAST_STATS["ks"] = ks_full
    LAST_STATS["new_lens"] = new_lens
    return out.astype(np.float32)


# revision 9
# speedup vs baseline: 1.6536x; 1.0859x over previous
"""Trainium2 Bass kernel for nn_CNN_RNN_88347477278730.

Pipeline (data-parallel over batch, 8 rows per core on 8 cores):
  kernel1 (device): chunked fp16 input projection (half hoisted, half
      interleaved into the early recurrence), then the 512-step
      select-policy GRUCell recurrence in full fp16 state with two 4-row
      batch streams for ILP; decisions batched into matmuls + is_gt at
      the end.
  host: compaction (gather kept tokens to the front), new_lens, Ldyn.
  kernel2 (device): compiled per dynamic sequence-length bucket t2
      (multiple of 32 >= max(new_lens)); chunked proj0, 2-layer GRU
      recurrence pipelined with a small lag, per-chunk proj1, Kim-CNN
      convs as shifted matmuls with compile-time pool windows, final
      linear.

All recurrence matmuls are gate-major (lhsT = weight tiles [K=128,
M=128], moving operand = h [K, batch]) so gate tensors land
partition-major where the elementwise engines are fast. The per-step
elementwise chain is 9 ops balanced across Vector/Scalar/GpSimd.
"""

import os
import subprocess
import sys
import tempfile

import numpy as np

# ---------------------------------------------------------------- constants
B, T, E, H, NF = 64, 512, 768, 256, 100
NCORES = 8
BPC = B // NCORES  # batch rows per core
KE = E // 128      # 6 K-tiles over the embedding dim
KH = H // 128      # 2 K-tiles over the hidden dim
GC = (3 * H) // 128  # 6 gate chunks (r: 0-1, z: 2-3, n: 4-5)

_F32 = None  # set lazily to mybir.dt.float32


# ------------------------------------------------------------- tile patch
def _apply_tile_patch():
    """This walrus build rejects >2 sem waits on one SP control instruction;
    split the TileContext tail drain into several drains of <=2 waits."""
    import concourse.tile as tile
    from concourse.vector_clock import ScopedClock, VectorClock

    if getattr(tile.TileContext, "_drain_split_patched", False):
        return

    def _patched(self, tick_clock, wait_clock):
        gc = tick_clock.global_clock
        n = len(gc)
        for start in range(0, n, 1):
            vec = [0] * n
            any_set = False
            for p in range(start, min(start + 1, n)):
                vec[p] = gc[p]
                any_set = any_set or vec[p] > 0
            if not any_set:
                continue
            d = self.nc.sync.drain()
            wait_clock.add_sem_waits(d.ins, ScopedClock({None: VectorClock(vec)}))
        self.nc.all_engine_barrier()
        assert self.sems is not None
        popped = self.nc._tile_sem_poison_stack.pop()
        assert popped is self._sem_poison
        self.nc.clear_and_free_semaphores(list(self.sems.allocated().values()))
        self.nc.all_engine_barrier()

    tile.TileContext._drain_and_barrier = _patched
    tile.TileContext._drain_split_patched = True


# ------------------------------------------------------------- gumbel (CPU)
def _gumbel_cpu():
    """jax.random.gumbel(key(42), (T-1, B, 2), f32) — computed in a CPU-jax
    subprocess so the accelerator backend is never involved (it must be
    bit-identical to the reference's CPU computation)."""
    path = os.path.join(tempfile.mkdtemp(), "gumbel.npy")
    code = (
        "import numpy as np, jax, jax.numpy as jnp\n"
        f"g = jax.random.gumbel(jax.random.key(42), ({T - 1}, {B}, 2), jnp.float32)\n"
        f"np.save({path!r}, np.asarray(g))\n"
    )
    env = dict(os.environ)
    env["TRN_TERMINAL_POOL_IPS"] = ""
    env["JAX_PLATFORMS"] = "cpu"
    extra = [p for p in sys.path if p and os.path.isdir(p)]
    env["PYTHONPATH"] = os.pathsep.join(extra)
    subprocess.run([sys.executable, "-c", code], env=env, check=True, capture_output=True)
    return np.load(path)


# ------------------------------------------------------------- host packing
def _pack_T(a2d):
    """[rows(=128*k), cols] -> [128, k, cols] weight-tile layout."""
    rows, cols = a2d.shape
    k = rows // 128
    return np.ascontiguousarray(a2d.reshape(k, 128, cols).transpose(1, 0, 2)).astype(np.float32)


def _pack_bias(b1d):
    """[128*k] -> [128, k]"""
    k = b1d.shape[0] // 128
    return np.ascontiguousarray(b1d.reshape(k, 128).T).astype(np.float32)


def _pack_embT(emb_rows, t_len=T):
    """[bpc, T, E] -> [KE, 128, bpc*T] (e-major tiles, free dims (b, t))."""
    bpc = emb_rows.shape[0]
    x = emb_rows.transpose(2, 0, 1).reshape(KE, 128, bpc * t_len)
    return np.ascontiguousarray(x).astype(np.float32)


def _pack_gru_weights(Wih, Whh, bih, bhh):
    """Returns (wihT, whhT, bias_proj, bhhn_rep) packings.

    bias_proj folds bih+bhh for the r,z chunks (added once at projection
    time); n chunks get bih only, with bhh_n applied per-step (it must be
    added to h@Whh_n *before* the r* multiply)."""
    wihT = _pack_T(np.ascontiguousarray(Wih.T))  # [128, KE or KH, 3H]
    whhT = _pack_T(np.ascontiguousarray(Whh.T))  # [128, KH, 3H]
    bias = np.empty(3 * H, np.float32)
    bias[: 2 * H] = bih[: 2 * H] + bhh[: 2 * H]
    bias[2 * H:] = bih[2 * H:]
    bias_proj = _pack_bias(bias)  # [128, GC]
    bhhn = _pack_bias(bhh[2 * H:])  # [128, KH]
    bhhn_rep = np.ascontiguousarray(
        np.broadcast_to(bhhn[:, :, None], (128, KH, BPC))
    ).astype(np.float32)
    return wihT, whhT, bias_proj, bhhn_rep


def _np_f16():
    from concourse import mybir

    return mybir.dt.np(mybir.dt.float16)


# ------------------------------------------------------------- bass builders
def _mk_nc():
    import concourse.bass as bass

    return bass.Bass("TRN2", target_bir_lowering=False, debug=False, num_devices=1)


def _split_excess_waits(nc, max_waits=1):
    """This walrus build can only encode ~2 sem waits per instruction
    (setupSyncWait 'Too many sync wait commands'). Hoist excess waits onto
    same-engine NoOps inserted just before the over-subscribed instruction;
    engine queues execute in order, so the wait semantics are identical."""
    from concourse import mybir

    nid = [0]
    for f in nc.m.functions:
        for bb in f.blocks:
            out = []
            changed = False
            for inst in bb.instructions:
                si = inst.sync_info
                lim = max_waits
                if si is not None and si.on_wait and len(si.on_wait) > lim:
                    waits = list(si.on_wait)
                    extra, keep = waits[:-lim], waits[-lim:]
                    for j in range(0, len(extra), max_waits):
                        nop = mybir.InstNoOp(
                            name=f"I-waitnop-{nid[0]}", ins=[], outs=[])
                        nid[0] += 1
                        nop.engine = inst.engine
                        nop.sync_info = mybir.SyncInfo(
                            on_wait=extra[j: j + max_waits], on_update=[])
                        nc.register_instruction(nop, overwrite=True)
                        out.append(nop)
                    inst.sync_info = mybir.SyncInfo(
                        on_wait=keep, on_update=list(si.on_update or []))
                    changed = True
                out.append(inst)
            if changed:
                bb.instructions = out
    return nc


def _proj_builder(nc, tc, misc_pool, dma_pool, big_ps, src_dram, wihT, biasc,
                  biasc_rep, gi, t_len, n_k, act, alu, f16, f32, tag):
    """Returns (emit_prefix, groups): chunked input projection.

    emit_prefix() emits the first t-half; `groups` is a list of closures,
    each emitting one (b, c) group of the second t-half (to be interleaved
    into the early recurrence steps). Copies alternate ACT (per-partition
    bias) / DVE (bias_rep tensor) to balance engines."""
    HALF = t_len // 2
    src_tiles = {}

    def _src(hb, b):
        key = (hb, b)
        if key not in src_tiles:
            s = dma_pool.tile([128, n_k, HALF], f16, tag=f"{tag}src")
            for k in range(n_k):
                nc.sync.dma_start(
                    s[:, k, :],
                    src_dram[k, :, b * t_len + hb * HALF: b * t_len + hb * HALF + HALF],
                )
            src_tiles[key] = s
        return src_tiles[key]

    def _group(hb, b, c):
        src = _src(hb, b)
        ps = big_ps.tile([128, HALF], f32, tag="bps")
        for k in range(n_k):
            nc.tensor.matmul(
                ps[:], wihT[:, k, c * 128: (c + 1) * 128], src[:, k, :],
                start=(k == 0), stop=(k == n_k - 1),
            )
        t0 = hb * HALF
        dst = gi[:, c, b, t0: t0 + HALF]
        if (b * GC + c) % 2 == 0:
            nc.scalar.activation(dst, ps[:], act.Identity, bias=biasc[:, c: c + 1])
        else:
            nc.vector.tensor_tensor(dst, ps[:], biasc_rep[:, c, :], alu.add)

    def emit_prefix():
        for b in range(BPC):
            for c in range(GC):
                _group(0, b, c)

    groups = [
        (lambda b=b, c=c: _group(1, b, c))
        for b in range(BPC) for c in range(GC)
    ]
    return emit_prefix, groups


def build_kernel1(t_len=T):
    """Select-policy kernel: fp16 everywhere; ONE merged 8-row stream (the
    per-step serial chain latency is the period — extra streams only add
    engine-queue coupling); gi_rz and bhh_n preloaded into PSUM off-chain so
    the gate matmuls accumulate straight onto them (start=False after a
    has_written-priming dummy matmul); 7-op chain; decisions batched at
    the end."""
    import concourse.tile as tile
    from concourse import mybir

    _apply_tile_patch()
    nc = _mk_nc()
    f32 = mybir.dt.float32
    f16 = mybir.dt.float16
    act = mybir.ActivationFunctionType
    alu = mybir.AluOpType
    HALF = t_len // 2

    embT_d = nc.dram_tensor("embT", [KE, 128, BPC * t_len], f16, kind="ExternalInput").ap()
    wihcT_d = nc.dram_tensor("wihcT", [128, KE, 3 * H], f16, kind="ExternalInput").ap()
    biasc_d = nc.dram_tensor("biasc", [128, GC], f32, kind="ExternalInput").ap()
    biascr_d = nc.dram_tensor("biascr", [128, GC, HALF], f32, kind="ExternalInput").ap()
    whh16_d = nc.dram_tensor("whh16", [128, KH, 3 * H], f16, kind="ExternalInput").ap()
    bhhnc_d = nc.dram_tensor("bhhnc", [128, KH, BPC], f32, kind="ExternalInput").ap()
    wdiffT_d = nc.dram_tensor("wdiffT", [128, KH, 1], f16, kind="ExternalInput").ap()
    ncdiff_d = nc.dram_tensor("ncdiff", [1, BPC * t_len], f32, kind="ExternalInput").ap()
    ks_d = nc.dram_tensor("ks", [1, BPC * t_len], f32, kind="ExternalOutput").ap()

    with tile.TileContext(nc) as tc:
        from contextlib import ExitStack

        with ExitStack() as ctx:
            wpool = ctx.enter_context(tc.tile_pool(name="weights", bufs=1))
            gipool = ctx.enter_context(tc.tile_pool(name="gi", bufs=1))
            hpool = ctx.enter_context(tc.tile_pool(name="hist", bufs=1))
            dma_pool = ctx.enter_context(tc.tile_pool(name="dma", bufs=2))
            big_ps = ctx.enter_context(tc.tile_pool(name="bigps", bufs=2, space="PSUM"))
            rz_psp = ctx.enter_context(tc.tile_pool(name="rzps", bufs=1, space="PSUM"))
            n_psp = ctx.enter_context(tc.tile_pool(name="nps", bufs=1, space="PSUM"))
            sb_pool = ctx.enter_context(tc.tile_pool(name="gates", bufs=2))
            misc = ctx.enter_context(tc.tile_pool(name="misc", bufs=1))

            def _load(pool, dram, shape, tag, dt=f32):
                t_ = pool.tile(shape, dt, tag=tag)
                nc.sync.dma_start(t_[:], dram[:])
                return t_

            wihcT = _load(wpool, wihcT_d, [128, KE, 3 * H], "wihcT", f16)
            biasc = _load(wpool, biasc_d, [128, GC], "biasc")
            biascr = _load(wpool, biascr_d, [128, GC, HALF], "biascr")
            whh16 = _load(wpool, whh16_d, [128, KH, 3 * H], "whh16", f16)
            bhhnc_rep = _load(wpool, bhhnc_d, [128, KH, BPC], "bhhnc")
            wdiffT = _load(misc, wdiffT_d, [128, KH, 1], "wdiffT", f16)
            ncdiff = _load(misc, ncdiff_d, [1, BPC * t_len], "ncdiff")
            zs = misc.tile([1, 128], f16, tag="zs")
            zx = misc.tile([1, 4 * BPC], f16, tag="zx")
            nc.vector.memset(zs[:], 0.0)
            nc.vector.memset(zx[:], 0.0)

            # gi layout: [128, GC, BPC, t] (t innermost: contiguous proj copies)
            gi = gipool.tile([128, GC, BPC, t_len], f16, tag="gi")
            emit_prefix, groups = _proj_builder(
                nc, tc, misc, dma_pool, big_ps, embT_d, wihcT, biasc, biascr,
                gi, t_len, KE, act, alu, f16, f32, "proj")
            emit_prefix()

            hist = hpool.tile([128, KH, BPC, t_len], f16, tag="hist")
            rz_ps = rz_psp.tile([128, 4, BPC], f32, tag="rz")
            n_ps = n_psp.tile([128, KH, BPC], f32, tag="n")

            # prime has_written for the preload+accumulate banks
            nc.tensor.matmul(rz_ps[:], zs[:], zx[:], start=True, stop=True)
            nc.tensor.matmul(n_ps[:], zs[:], zx[0:1, 0: KH * BPC], start=True, stop=True)
            # initial preloads for t=0
            nc.vector.tensor_copy(rz_ps[:], gi[:, 0:4, :, 0])
            nc.scalar.activation(n_ps[:], bhhnc_rep[:], act.Identity)

            def _gh(w, rhs, last):
                """Accumulate Whh @ rhs into the gate banks (start=False)."""
                for c in (0, 1, 2, 3, 4, 5):
                    dst = rz_ps[:, c, :] if c < 4 else n_ps[:, c - 4, :]
                    for k in range(KH):
                        nc.tensor.matmul(
                            dst, w[:, k, c * 128: (c + 1) * 128], rhs[:, k, :],
                            start=False, stop=(last and k == KH - 1),
                            skip_group_check=True,
                        )

            def emit_step(t):
                # gh(t) was already accumulated by step t-1 (MM-split: the
                # Whh@nn part right after tanh, the Whh@d part after op8).
                # Chain: sigma_r -> x r -> +gi_n -> tanh -> (h_prev-nn) ->
                # x z -> MM_d; sigma_z, preloads, MM_nn, h-write off-chain.
                rz = sb_pool.tile([128, 4, BPC], f32, tag="rz")
                tmp = sb_pool.tile([128, KH, BPC], f32, tag="tmp")
                nc.scalar.activation(rz[:, 0:2, :], rz_ps[:, 0:2, :], act.Sigmoid)
                nc.scalar.activation(rz[:, 2:4, :], rz_ps[:, 2:4, :], act.Sigmoid)
                nc.vector.tensor_tensor(tmp[:], n_ps[:], rz[:, 0:KH, :], alu.mult)
                nc.vector.tensor_tensor(tmp[:], tmp[:], gi[:, 4:GC, :, t], alu.add)
                if t + 1 < t_len:
                    nc.vector.tensor_copy(rz_ps[:], gi[:, 0:4, :, t + 1])
                nn16 = sb_pool.tile([128, KH, BPC], f16, tag="nn")
                nc.scalar.activation(nn16[:], tmp[:], act.Tanh)
                if t + 1 < t_len:
                    nc.scalar.activation(n_ps[:], bhhnc_rep[:], act.Identity)
                    _gh(whh16, nn16, last=False)
                d = sb_pool.tile([128, KH, BPC], f32, tag="dd")
                if t > 0:
                    nc.vector.tensor_tensor(d[:], hist[:, :, :, t - 1], nn16[:], alu.subtract)
                else:
                    nc.vector.tensor_scalar(d[:], nn16[:], -1.0, None, alu.mult)
                d16 = sb_pool.tile([128, KH, BPC], f16, tag="dd16")
                nc.vector.tensor_tensor(d16[:], rz[:, 2:4, :], d[:], alu.mult)
                if t + 1 < t_len:
                    _gh(whh16, d16, last=True)
                nc.gpsimd.tensor_tensor(hist[:, :, :, t], nn16[:], d16[:], alu.add)

            gidx = 0
            for t in range(t_len):
                emit_step(t)
                if t % 2 == 1 and gidx < len(groups):
                    groups[gidx]()
                    gidx += 1
            while gidx < len(groups):
                groups[gidx]()
                gidx += 1

            # ---- batched decisions: ks[b,t] = (h_t . wdiff > ncdiff) ----
            ks_sb = misc.tile([1, BPC * t_len], f32, tag="kssb")
            for b in range(BPC):
                dps = big_ps.tile([1, t_len], f32, tag="bps")
                for k in range(KH):
                    nc.tensor.matmul(
                        dps[:], wdiffT[:, k, :], hist[:, k, b, :],
                        start=(k == 0), stop=(k == KH - 1),
                    )
                nc.vector.tensor_tensor(
                    ks_sb[0:1, b * t_len: (b + 1) * t_len], dps[:],
                    ncdiff[0:1, b * t_len: (b + 1) * t_len], alu.is_gt,
                )
            nc.sync.dma_start(ks_d[:], ks_sb[:])

    return _split_excess_waits(nc)


def build_kernel2(t2, kf3, kf4, kf5):
    """GRU0/GRU1 + convs + pooling + final linear at dynamic length t2.

    Both layers use the k1-style low-latency step (PSUM preloads, split
    sigma, DVE-resident chain); no matmul split (two chains share the PE,
    so the per-wave PE budget matters more than each chain's MM segment).
    proj1 computed per-D-chunk from the o1 history into a gi1 buffer;
    conv max-pool windows (kf*) are compile-time constants."""
    import concourse.tile as tile
    from concourse import mybir

    _apply_tile_patch()
    nc = _mk_nc()
    f32 = mybir.dt.float32
    f16 = mybir.dt.float16
    act = mybir.ActivationFunctionType
    alu = mybir.AluOpType
    D = 32
    LAG = D + 8
    HALF = t2 // 2

    nembT_d = nc.dram_tensor("nembT", [KE, 128, BPC * t2], f16, kind="ExternalInput").ap()
    wih0T_d = nc.dram_tensor("wih0T", [128, KE, 3 * H], f16, kind="ExternalInput").ap()
    whh0T_d = nc.dram_tensor("whh0T", [128, KH, 3 * H], f16, kind="ExternalInput").ap()
    bias0_d = nc.dram_tensor("bias0", [128, GC], f32, kind="ExternalInput").ap()
    bias0r_d = nc.dram_tensor("bias0r", [128, GC, HALF], f32, kind="ExternalInput").ap()
    bhhn0_d = nc.dram_tensor("bhhn0", [128, KH, BPC], f32, kind="ExternalInput").ap()
    wih1T_d = nc.dram_tensor("wih1T", [128, KH, 3 * H], f16, kind="ExternalInput").ap()
    whh1T_d = nc.dram_tensor("whh1T", [128, KH, 3 * H], f16, kind="ExternalInput").ap()
    bias1_d = nc.dram_tensor("bias1", [128, GC], f32, kind="ExternalInput").ap()
    bias1r_d = nc.dram_tensor("bias1r", [128, GC, D], f32, kind="ExternalInput").ap()
    bhhn1_d = nc.dram_tensor("bhhn1", [128, KH, BPC], f32, kind="ExternalInput").ap()
    vt_d = nc.dram_tensor("vt", [1, BPC * t2], f16, kind="ExternalInput").ap()
    cw_d = nc.dram_tensor("cw", [128, 12, KH, NF], f16, kind="ExternalInput").ap()
    cb_d = nc.dram_tensor("cb", [NF, 3], f32, kind="ExternalInput").ap()
    woutT_d = nc.dram_tensor("woutT", [NF, 3], f32, kind="ExternalInput").ap()
    bout_d = nc.dram_tensor("bout", [1, 1], f32, kind="ExternalInput").ap()
    out_d = nc.dram_tensor("out", [1, BPC], f32, kind="ExternalOutput").ap()

    FS = (3, 4, 5)
    KFS = (kf3, kf4, kf5)

    with tile.TileContext(nc) as tc:
        from contextlib import ExitStack

        with ExitStack() as ctx:
            wpool = ctx.enter_context(tc.tile_pool(name="weights", bufs=1))
            gipool = ctx.enter_context(tc.tile_pool(name="gi", bufs=1))
            opool = ctx.enter_context(tc.tile_pool(name="obuf", bufs=1))
            dma_pool = ctx.enter_context(tc.tile_pool(name="dma", bufs=2))
            big_ps = ctx.enter_context(tc.tile_pool(name="bigps", bufs=2, space="PSUM"))
            rz0_psp = ctx.enter_context(tc.tile_pool(name="rz0ps", bufs=1, space="PSUM"))
            n0_psp = ctx.enter_context(tc.tile_pool(name="n0ps", bufs=1, space="PSUM"))
            rz1_psp = ctx.enter_context(tc.tile_pool(name="rz1ps", bufs=1, space="PSUM"))
            n1_psp = ctx.enter_context(tc.tile_pool(name="n1ps", bufs=1, space="PSUM"))
            sb_pool = ctx.enter_context(tc.tile_pool(name="gates", bufs=2))
            misc = ctx.enter_context(tc.tile_pool(name="misc", bufs=1))

            def _load(pool, dram, shape, tag, dt=f32):
                t_ = pool.tile(shape, dt, tag=tag)
                nc.sync.dma_start(t_[:], dram[:])
                return t_

            wih0T = _load(wpool, wih0T_d, [128, KE, 3 * H], "wih0", f16)
            whh0T = _load(wpool, whh0T_d, [128, KH, 3 * H], "whh0", f16)
            bias0 = _load(wpool, bias0_d, [128, GC], "bias0")
            bias0r = _load(wpool, bias0r_d, [128, GC, HALF], "bias0r")
            bhhn0 = _load(wpool, bhhn0_d, [128, KH, BPC], "bhhn0")
            wih1T = _load(wpool, wih1T_d, [128, KH, 3 * H], "wih1", f16)
            whh1T = _load(wpool, whh1T_d, [128, KH, 3 * H], "whh1", f16)
            bias1 = _load(wpool, bias1_d, [128, GC], "bias1")
            bias1r = _load(wpool, bias1r_d, [128, GC, D], "bias1r")
            bhhn1 = _load(wpool, bhhn1_d, [128, KH, BPC], "bhhn1")
            cw = _load(wpool, cw_d, [128, 12, KH, NF], "cw", f16)
            cb = _load(misc, cb_d, [NF, 3], "cb")
            woutT = _load(misc, woutT_d, [NF, 3], "woutT")
            bout = _load(misc, bout_d, [1, 1], "bout")
            vt = _load(misc, vt_d, [1, BPC * t2], "vt", f16)
            zs = misc.tile([1, 128], f16, tag="zs")
            zx = misc.tile([1, 4 * BPC], f16, tag="zx")
            nc.vector.memset(zs[:], 0.0)
            nc.vector.memset(zx[:], 0.0)

            gi0 = gipool.tile([128, GC, BPC, t2], f16, tag="gi0")
            gi1 = gipool.tile([128, GC, BPC, t2], f16, tag="gi1")
            o1 = opool.tile([128, KH, BPC, t2], f16, tag="o1")
            o2 = opool.tile([128, KH, BPC, t2], f16, tag="o2")

            emit_prefix, groups = _proj_builder(
                nc, tc, misc, dma_pool, big_ps, nembT_d, wih0T, bias0, bias0r,
                gi0, t2, KE, act, alu, f16, f32, "proj0")
            emit_prefix()

            rz0 = rz0_psp.tile([128, 4, BPC], f32, tag="rz0")
            n0 = n0_psp.tile([128, KH, BPC], f32, tag="n0")
            rz1 = rz1_psp.tile([128, 4, BPC], f32, tag="rz1")
            n1 = n1_psp.tile([128, KH, BPC], f32, tag="n1")

            for ps_t in (rz0, rz1):
                nc.tensor.matmul(ps_t[:], zs[:], zx[:], start=True, stop=True)
            for ps_t in (n0, n1):
                nc.tensor.matmul(ps_t[:], zs[:], zx[0:1, 0: KH * BPC], start=True, stop=True)
            # initial preloads
            nc.vector.tensor_copy(rz0[:], gi0[:, 0:4, :, 0])
            nc.scalar.activation(n0[:], bhhn0[:], act.Identity)
            nc.scalar.activation(n1[:], bhhn1[:], act.Identity)

            def emit_step(t, gi, whh, bhhn, hist, prev, rz_ps, n_ps, sfx, tl):
                """One GRU layer step; hist[t] <- GRU(prev[t-1], gi[t]).
                prev is the layer input history tile holding h (== hist)."""
                if t > 0:
                    h_prev = hist[:, :, :, t - 1]
                    for c in (0, 1, 2, 3, 4, 5):
                        dst = rz_ps[:, c, :] if c < 4 else n_ps[:, c - 4, :]
                        for k in range(KH):
                            nc.tensor.matmul(
                                dst, whh[:, k, c * 128: (c + 1) * 128],
                                h_prev[:, k, :],
                                start=False, stop=(k == KH - 1),
                                skip_group_check=True,
                            )
                rz = sb_pool.tile([128, 4, BPC], f32, tag="rz" + sfx)
                tmp = sb_pool.tile([128, KH, BPC], f32, tag="tmp" + sfx)
                nc.scalar.activation(rz[:, 0:2, :], rz_ps[:, 0:2, :], act.Sigmoid)
                nc.scalar.activation(rz[:, 2:4, :], rz_ps[:, 2:4, :], act.Sigmoid)
                nc.vector.tensor_tensor(tmp[:], n_ps[:], rz[:, 0:KH, :], alu.mult)
                nc.vector.tensor_tensor(tmp[:], tmp[:], gi[:, 4:GC, :, t], alu.add)
                if t + 1 < tl:
                    nc.vector.tensor_copy(rz_ps[:], gi[:, 0:4, :, t + 1])
                nn16 = sb_pool.tile([128, KH, BPC], f16, tag="nn" + sfx)
                nc.scalar.activation(nn16[:], tmp[:], act.Tanh)
                if t + 1 < tl:
                    nc.scalar.activation(n_ps[:], bhhn[:], act.Identity)
                d = sb_pool.tile([128, KH, BPC], f32, tag="dd" + sfx)
                if t > 0:
                    nc.vector.tensor_tensor(d[:], hist[:, :, :, t - 1], nn16[:], alu.subtract)
                else:
                    nc.vector.tensor_scalar(d[:], nn16[:], -1.0, None, alu.mult)
                d16 = sb_pool.tile([128, KH, BPC], f16, tag="d6" + sfx)
                nc.vector.tensor_tensor(d16[:], rz[:, 2:4, :], d[:], alu.mult)
                nc.gpsimd.tensor_tensor(hist[:, :, :, t], nn16[:], d16[:], alu.add)

            def emit_proj1_chunk(ci):
                t0, t1 = ci * D, (ci + 1) * D
                for b in range(BPC):
                    for c in range(GC):
                        ps = big_ps.tile([128, D], f32, tag="bps")
                        for k in range(KH):
                            nc.tensor.matmul(
                                ps[:], wih1T[:, k, c * 128: (c + 1) * 128],
                                o1[:, k, b, t0:t1],
                                start=(k == 0), stop=(k == KH - 1),
                            )
                        dst = gi1[:, c, b, t0:t1]
                        if (b * GC + c) % 2 == 0:
                            nc.scalar.activation(
                                dst, ps[:], act.Identity, bias=bias1[:, c: c + 1])
                        else:
                            nc.vector.tensor_tensor(dst, ps[:], bias1r[:, c, :], alu.add)

            l1_started = [False]
            gidx = 0
            for w in range(t2 + LAG):
                if w < t2:
                    emit_step(w, gi0, whh0T, bhhn0, o1, o1, rz0, n0, "0", t2)
                    if w % 2 == 1 and gidx < len(groups):
                        groups[gidx]()
                        gidx += 1
                if w >= LAG:
                    t = w - LAG
                    if not l1_started[0]:
                        nc.vector.tensor_copy(rz1[:], gi1[:, 0:4, :, 0])
                        l1_started[0] = True
                    emit_step(t, gi1, whh1T, bhhn1, o2, o2, rz1, n1, "1", t2)
                if w < t2 and w % D == D - 1:
                    emit_proj1_chunk(w // D)
            while gidx < len(groups):
                groups[gidx]()
                gidx += 1

            # ---- zero o2 past new_lens: o2 *= vt ----
            # partition-broadcast vt via a K=1 ones-matmul (PE outer product)
            ones_sb = misc.tile([1, 128], f16, tag="ones")
            nc.vector.memset(ones_sb[:], 1.0)
            for b in range(BPC):
                vtb = big_ps.tile([128, t2], f32, tag="bps")
                nc.tensor.matmul(
                    vtb[:], ones_sb[:], vt[0:1, b * t2: (b + 1) * t2],
                    start=True, stop=True,
                )
                for k in range(KH):
                    nc.vector.tensor_tensor(
                        o2[:, k, b, :], o2[:, k, b, :], vtb[:], alu.mult
                    )

            # ---- convs + relu + max-pool over compile-time window ----
            pooled = misc.tile([NF, 3, BPC], f32, tag="pooled")
            for b in range(BPC):
                for fi, fs in enumerate(FS):
                    nw = t2 - fs + 1
                    kf = KFS[fi]
                    ps = big_ps.tile([NF, t2], f32, tag="bps")
                    m0 = sum(FS[:fi])  # flat (fs,dt) base index
                    first = True
                    for dt_ in range(fs):
                        for k in range(KH):
                            nc.tensor.matmul(
                                ps[:, :nw],
                                cw[:, m0 + dt_, k, :],
                                o2[:, k, b, dt_: dt_ + nw],
                                start=first,
                                stop=(dt_ == fs - 1 and k == KH - 1),
                            )
                            first = False
                    crelu = sb_pool.tile([NF, t2], f32, tag="crelu")
                    nc.scalar.activation(
                        crelu[:, :kf], ps[:, :kf], act.Relu, bias=cb[:, fi: fi + 1]
                    )
                    nc.vector.tensor_reduce(
                        pooled[:, fi, b: b + 1], crelu[:, :kf],
                        mybir.AxisListType.X, alu.max,
                    )

            # ---- final linear ----
            fps = big_ps.tile([1, BPC], f32, tag="bps")
            for fi in range(3):
                nc.tensor.matmul(
                    fps[:], woutT[:, fi: fi + 1], pooled[:, fi, :],
                    start=(fi == 0), stop=(fi == 2),
                )
            out_sb = misc.tile([1, BPC], f32, tag="outsb")
            nc.scalar.activation(out_sb[:], fps[:], act.Identity, bias=bout[0:1, 0:1])
            nc.sync.dma_start(out_d[:], out_sb[:])

    return _split_excess_waits(nc)


# ------------------------------------------------------------- host orchestration
def _host_pack_k1(inputs, gumbel, t_len=T):
    f16 = _np_f16()
    emb = np.asarray(inputs["embedded"], np.float32)
    mask = np.asarray(inputs["mask"])
    lens = mask.sum(1)
    maxlen = int(lens.max())

    wihcT, whhT, biasc, bhhnc = _pack_gru_weights(
        inputs["Wih_c"], inputs["Whh_c"], inputs["bih_c"], inputs["bhh_c"])
    wdiff = (inputs["Wsel"][1] - inputs["Wsel"][0]).astype(np.float32)
    wdiffT = np.ascontiguousarray(wdiff.reshape(KH, 128).T[:, :, None])
    bdiff = float(inputs["bsel"][1] - inputs["bsel"][0])

    # ncdiff[b, t]: k_t = (h.wdiff > ncdiff); forced off when t >= maxlen-1
    ncdiff = np.full((B, t_len), 1.0e30, np.float32)
    upto = min(maxlen - 1, t_len)
    for t in range(1, upto):
        ncdiff[:, t] = -(bdiff + gumbel[t - 1, :, 1] - gumbel[t - 1, :, 0])

    biascr = np.ascontiguousarray(
        np.broadcast_to(biasc[:, :, None], (128, GC, t_len // 2))).astype(np.float32)

    in_maps = []
    for c in range(NCORES):
        rows = slice(c * BPC, (c + 1) * BPC)
        in_maps.append({
            "embT": _pack_embT(emb[rows, :t_len], t_len).astype(f16),
            "wihcT": wihcT.astype(f16),
            "biasc": biasc,
            "biascr": biascr,
            "whh16": whhT.astype(f16),
            "bhhnc": bhhnc,
            "wdiffT": wdiffT.astype(f16),
            "ncdiff": np.ascontiguousarray(
                ncdiff[rows].reshape(1, BPC * t_len)),
        })
    return in_maps, lens, maxlen


def _host_compact(inputs, ks_full, lens, maxlen, t_len=T):
    """ks_full: [B, t_len] decision bits (row t=0 ignored; selected[:,0]=1)."""
    emb = np.asarray(inputs["embedded"], np.float32)
    selected = np.zeros((B, t_len), np.int64)
    selected[:, 0] = 1
    selected[:, 1:] = ks_full[:, 1:]
    pos = np.arange(t_len)
    sel_valid = np.where(pos[None, :] < (lens - 1)[:, None], selected, 0)
    new_mask = np.where(pos[None, :] == (lens - 1)[:, None], 1, sel_valid)
    new_lens = new_mask.sum(1)
    Ldyn = max(int(new_lens.max()), 7)

    t2 = max(-(-Ldyn // 64) * 64, 64)
    new_emb = np.zeros((B, t2, E), np.float32)
    for b in range(B):
        idx = np.nonzero(new_mask[b])[0]
        new_emb[b, : len(idx)] = emb[b, idx]
    return new_emb, new_lens, Ldyn, t2


def _host_pack_k2(inputs, new_emb, new_lens, Ldyn, t2):
    f16 = _np_f16()
    wih0T, whh0T, bias0, bhhn0 = _pack_gru_weights(
        inputs["Wih0"], inputs["Whh0"], inputs["bih0"], inputs["bhh0"])
    wih1T, whh1T, bias1, bhhn1 = _pack_gru_weights(
        inputs["Wih1"], inputs["Whh1"], inputs["bih1"], inputs["bhh1"])

    FS = (3, 4, 5)
    cw = np.zeros((128, 12, KH, NF), np.float32)
    cb = np.zeros((NF, 3), np.float32)
    m = 0
    for fi, fs in enumerate(FS):
        w = np.asarray(inputs[f"conv_w{fs}"], np.float32)  # [NF,1,fs,H]
        cb[:, fi] = np.asarray(inputs[f"conv_b{fs}"], np.float32)
        for dt_ in range(fs):
            wt = w[:, 0, dt_, :].T  # [H, NF]
            cw[:, m, :, :] = wt.reshape(KH, 128, NF).transpose(1, 0, 2)
            m += 1

    woutT = np.ascontiguousarray(
        np.asarray(inputs["Wout"], np.float32)[0].reshape(3, NF).T)
    bout = np.asarray(inputs["bout"], np.float32).reshape(1, 1)

    vt_full = (np.arange(t2)[None, :] < new_lens[:, None]).astype(np.float32)

    bias0r = np.ascontiguousarray(
        np.broadcast_to(bias0[:, :, None], (128, GC, t2 // 2))).astype(np.float32)
    bias1r = np.ascontiguousarray(
        np.broadcast_to(bias1[:, :, None], (128, GC, 32))).astype(np.float32)

    in_maps = []
    for c in range(NCORES):
        rows = slice(c * BPC, (c + 1) * BPC)
        in_maps.append({
            "nembT": _pack_embT(new_emb[rows], t2).astype(f16),
            "wih0T": wih0T.astype(f16), "whh0T": whh0T.astype(f16),
            "bias0": bias0, "bias0r": bias0r, "bhhn0": bhhn0,
            "wih1T": wih1T.astype(f16), "whh1T": whh1T.astype(f16),
            "bias1": bias1, "bias1r": bias1r, "bhhn1": bhhn1,
            "vt": np.ascontiguousarray(
                vt_full[rows].reshape(1, BPC * t2)).astype(f16),
            "cw": cw.astype(f16), "cb": cb,
            "woutT": woutT, "bout": bout,
        })
    return in_maps


_NC_CACHE = {}


def _get_nc1(t_len=T):
    key = (1, t_len)
    if key not in _NC_CACHE:
        _NC_CACHE[key] = build_kernel1(t_len)
    return _NC_CACHE[key]


def _get_nc2(t2, kfs):
    key = (2, t2, kfs)
    if key not in _NC_CACHE:
        _NC_CACHE[key] = build_kernel2(t2, *kfs)
    return _NC_CACHE[key]


TRACE = False  # set True (with an NTFF hook registered) to collect exec times
LAST_STATS = {}


def kernel(**inputs):
    from concourse import bass_utils

    gumbel = _gumbel_cpu()
    core_ids = list(range(NCORES))

    in_maps1, lens, maxlen = _host_pack_k1(inputs, gumbel)
    nc1 = _get_nc1()
    res1 = bass_utils.run_bass_kernel_spmd(nc1, in_maps1, core_ids, trace=TRACE)
    ks_full = np.concatenate(
        [res1.results[c]["ks"].reshape(BPC, T) for c in range(NCORES)], axis=0)

    new_emb, new_lens, Ldyn, t2 = _host_compact(inputs, ks_full, lens, maxlen)
    kfs = tuple(min(Ldyn - fs + 1, t2 - fs + 1) for fs in (3, 4, 5))
    in_maps2 = _host_pack_k2(inputs, new_emb, new_lens, Ldyn, t2)
    nc2 = _get_nc2(t2, kfs)
    res2 = bass_utils.run_bass_kernel_spmd(nc2, in_maps2, core_ids, trace=TRACE)
    out = np.concatenate([res2.results[c]["out"][0] for c in range(NCORES)], axis=0)
    LAST_STATS["k1_ns"] = res1.exec_time_ns
    LAST_STATS["k2_ns"] = res2.exec_time_ns
    LAST_STATS["ks"] = ks_full
    LAST_STATS["new_lens"] = new_lens
    return out.astype(np.float32)
